# revision 17
# baseline (speedup 1.0000x reference)
"""AxialCrossMamba Trainium2 kernel.

Sharding: 8 cores = 4 directions x 2 batch-halves. Each core runs one
direction's Mamba block (its own weights) over two batches. Host does the
direction permutations (row/col/diag/anti, c-major [C, L] token layouts),
and the final 4-direction sigmoid gate.

Device pipeline per (job = one batch):
  in-proj matmul (bf16 PE) -> causal depthwise conv (PE) + fused silu (ACT)
  -> x-proj/dt matmuls (PE) -> fused softplus (ACT)
  -> selective scan: a = exp(dt*A) per-state-column ACT activations (fp32),
     b = u*B (bf16), tensor_tensor_scan over flattened (s,t) with
     trailing reset/hold boundary columns (even-width blocks keep the
     DVE 2x perf mode); scans split between Vector and GpSimd engines;
     h*C + tree reduce over s on Vector
  -> y = ys + xs*D (bf16), gate silu(z), out-proj matmul (bf16 PE).
"""

import sys

for _p in ("/opt/trn_rl_repo", "/root/.axon_site/_ro/trn_rl_repo"):
    if _p not in sys.path:
        sys.path.insert(0, _p)

from contextlib import ExitStack

import numpy as np
import ml_dtypes

import concourse.bass as bass
from concourse import bacc
import concourse.mybir as mybir
import concourse.tile as tile
from concourse.bass_utils import run_bass_kernel_spmd

BF16 = ml_dtypes.bfloat16

# Problem constants
B_, C_, H_, W_ = 4, 192, 64, 64
L = H_ * W_          # 4096 tokens
DS, DC = 16, 4       # d_state, d_conv
DI = 2 * C_          # 384 d_inner
DTR = (C_ + 15) // 16  # 12 dt_rank
NB = 2               # batches per core
ND = DI // 128       # 3 d-blocks
N_CORES = 8

GP_SBS = ()          # scans: Vector only (scan opcode invalid on Pool/GpSimd)
GP_B = False         # b = u*B on Vector (1x rate is cheap there)
GP_HCM = True        # h*C and first tree level on the Pool engine

AF = mybir.ActivationFunctionType
ALU = mybir.AluOpType
FP32 = mybir.dt.float32
BF = mybir.dt.bfloat16


def build_nc(L=L, TC=512, SB=4):
    """Build the SPMD single-core program (identical on all 8 cores)."""
    nc = bacc.Bacc("TRN2", debug=False)

    # ---- DRAM I/O ----
    tokT = nc.dram_tensor("tokT", [NB, C_, L], BF, kind="ExternalInput").ap()
    Win = nc.dram_tensor("Win", [C_, 2 * DI], BF, kind="ExternalInput").ap()
    convd = nc.dram_tensor("convd", [ND, DC, 128, 128], BF, kind="ExternalInput").ap()
    convb = nc.dram_tensor("convb", [DI, 1], FP32, kind="ExternalInput").ap()
    Wx = nc.dram_tensor("Wx", [DI, 96], BF, kind="ExternalInput").ap()
    Wdt = nc.dram_tensor("Wdt", [DTR, DI], BF, kind="ExternalInput").ap()
    bdt = nc.dram_tensor("bdt", [DI, 1], FP32, kind="ExternalInput").ap()
    Acoef = nc.dram_tensor("Acoef", [DI, DS], FP32, kind="ExternalInput").ap()
    Dsk = nc.dram_tensor("Dsk", [DI, 1], FP32, kind="ExternalInput").ap()
    Wout = nc.dram_tensor("Wout", [DI, C_], BF, kind="ExternalInput").ap()
    outT = nc.dram_tensor("outT", [NB, C_, L], FP32, kind="ExternalOutput").ap()
    # scratch
    z_scr = nc.dram_tensor("z_scr", [NB, ND, 128, L], BF, kind="Internal").ap()
    y_scr = nc.dram_tensor("y_scr", [NB, ND, 128, L], BF, kind="Internal").ap()
    bc_scr = nc.dram_tensor("bc_scr", [NB, 2, L // TC, DS * TC], BF, kind="Internal").ap()

    io = dict(tokT=tokT, Win=Win, convd=convd, convb=convb, Wx=Wx, Wdt=Wdt,
              bdt=bdt, Acoef=Acoef, Dsk=Dsk, Wout=Wout, outT=outT,
              z_scr=z_scr, y_scr=y_scr, bc_scr=bc_scr)
    with tile.TileContext(nc) as tc:
        with ExitStack() as ctx:
            _emit(ctx, tc, nc, io, L=L, TC=TC, SB=SB)
    nc.compile()
    return nc


def _emit(ctx, tc, nc, io, *, L, TC, SB):
    tokT, Win, convd, convb, Wx, Wdt, bdt = (
        io["tokT"], io["Win"], io["convd"], io["convb"], io["Wx"], io["Wdt"],
        io["bdt"])
    Acoef, Dsk, Wout, outT = io["Acoef"], io["Dsk"], io["Wout"], io["outT"]
    z_scr, y_scr, bc_scr = io["z_scr"], io["y_scr"], io["bc_scr"]

    P = 128
    NCH = L // TC          # t-chunks
    NSB = DS // SB         # s-blocks
    NN = max(1, L // 512)  # matmul n-chunks
    NSZ = L // NN
    TB = TC + 2            # scan block width (data + reset + hold columns)

    # ---- pools ----
    wp = ctx.enter_context(tc.tile_pool(name="weights", bufs=1))
    big = ctx.enter_context(tc.tile_pool(name="big", bufs=4))    # bf16 [128,L]
    af32 = ctx.enter_context(tc.tile_pool(name="af32", bufs=2))  # fp32 scan a
    hbf = ctx.enter_context(tc.tile_pool(name="hbf", bufs=3))    # bf16 scan h
    bcp = ctx.enter_context(tc.tile_pool(name="bcp", bufs=3))    # brep/crep
    bcls = ctx.enter_context(tc.tile_pool(name="bcls", bufs=2))  # b_/hcm
    dtp = ctx.enter_context(tc.tile_pool(name="dtp", bufs=1))    # dt bf16 resident
    xsp = ctx.enter_context(tc.tile_pool(name="xsp", bufs=1))    # xs bf16 resident
    sm = ctx.enter_context(tc.tile_pool(name="small", bufs=2))
    smE = ctx.enter_context(tc.tile_pool(name="smallE", bufs=2))
    pp = ctx.enter_context(tc.tile_pool(name="psum", bufs=2, space="PSUM"))
    pp2 = ctx.enter_context(tc.tile_pool(name="psum2", bufs=2, space="PSUM"))

    # ---- load weights ----
    win0 = wp.tile([P, 2 * DI], BF, tag="win0")
    win1 = wp.tile([C_ - P, 2 * DI], BF, tag="win1")
    nc.sync.dma_start(win0[:], Win[0:P, :])
    nc.sync.dma_start(win1[:], Win[P:C_, :])
    wdt_full = wp.tile([DTR, DI], BF, tag="wdt")
    nc.sync.dma_start(wdt_full[:], Wdt[:])
    wxs, cw3, cb3, bdt3, ac3, dsk3, wo3 = [], [], [], [], [], [], []
    for db in range(ND):
        r = slice(db * P, (db + 1) * P)
        w1 = wp.tile([P, 96], BF, tag=f"wx{db}")
        nc.sync.dma_start(w1[:], Wx[r, :]); wxs.append(w1)
        wconv = []
        for k in range(DC):
            wck = wp.tile([P, P], BF, tag=f"cw{db}_{k}", name=f"cw{db}_{k}")
            nc.sync.dma_start(wck[:], convd[db, k])
            wconv.append(wck)
        cw3.append(wconv)
        w3 = wp.tile([P, 1], FP32, tag=f"cb{db}")
        nc.sync.dma_start(w3[:], convb[r, :]); cb3.append(w3)
        w4 = wp.tile([P, 1], FP32, tag=f"bdt{db}")
        nc.sync.dma_start(w4[:], bdt[r, :]); bdt3.append(w4)
        w5 = wp.tile([P, DS], FP32, tag=f"ac{db}")
        nc.sync.dma_start(w5[:], Acoef[r, :]); ac3.append(w5)
        w6 = wp.tile([P, 1], FP32, tag=f"dsk{db}")
        nc.sync.dma_start(w6[:], Dsk[r, :]); dsk3.append(w6)
        w7 = wp.tile([P, C_], BF, tag=f"wo{db}")
        nc.sync.dma_start(w7[:], Wout[r, :]); wo3.append(w7)

    for j in range(NB):
        # ================= A: in-proj (+ conv interleaved) =================
        tok0 = big.tile([P, L], BF, tag="big")
        tok1 = big.tile([C_ - P, L], BF, tag="big")
        nc.sync.dma_start(tok0[:], tokT[j, 0:P, :])
        nc.sync.dma_start(tok1[:], tokT[j, P:C_, :])

        xs = []
        for m in range(2 * DI // P):   # M-blocks of xz^T; 0..2 -> xi, 3..5 -> z
            if m < ND:
                xi = big.tile([P, L + DC], BF, tag="big")
                nc.scalar.memzero(xi[:, 0:DC])
            mm = slice(m * P, (m + 1) * P)
            for n in range(NN):
                ns = slice(n * NSZ, (n + 1) * NSZ)
                ps = pp.tile([P, NSZ], FP32, tag="ps")
                nc.tensor.matmul(ps[:], win0[:, mm], tok0[:, ns],
                                 start=True, stop=False)
                nc.tensor.matmul(ps[:], win1[:, mm], tok1[:, ns],
                                 start=False, stop=True)
                if m < ND:
                    nc.scalar.copy(xi[:, DC + n * NSZ: DC + (n + 1) * NSZ],
                                   ps[:])
                else:
                    zt = smE.tile([P, NSZ], BF, tag="ztmp", bufs=2)
                    nc.scalar.activation(zt[:], ps[:], AF.Silu)
                    nc.sync.dma_start(z_scr[j, m - ND, :, ns], zt[:])
            if m < ND:
                # conv on PE via diagonal weight matrices, then fused silu
                db = m
                x_ = xsp.tile([P, L], BF, tag=f"xs{db}")
                for n in range(NN):
                    ns = slice(n * NSZ, (n + 1) * NSZ)
                    psc = pp.tile([P, NSZ], FP32, tag="psc")
                    for k in range(DC):
                        nc.tensor.matmul(
                            psc[:], cw3[db][k][:],
                            xi[:, 1 + k + n * NSZ: 1 + k + n * NSZ + NSZ],
                            start=(k == 0), stop=(k == DC - 1))
                    nc.scalar.activation(x_[:, ns], psc[:], AF.Silu,
                                         bias=cb3[db])
                xs.append(x_)

        # ================= C: dbc, dt =================
        dtl = sm.tile([DTR, L], BF, tag="dtl", bufs=1)
        for n in range(NN):
            ns = slice(n * NSZ, (n + 1) * NSZ)
            psd = pp2.tile([96, NSZ], FP32, tag="psd")
            for db in range(ND):
                nc.tensor.matmul(psd[:], wxs[db][:], xs[db][:, ns],
                                 start=(db == 0), stop=(db == ND - 1))
            nc.scalar.copy(dtl[:, ns], psd[0:DTR, :])
            bt = smE.tile([DS, NSZ], BF, tag="bct")
            ct = smE.tile([DS, NSZ], BF, tag="bct")
            nc.scalar.copy(bt[:], psd[32:32 + DS, :])
            nc.scalar.copy(ct[:], psd[64:64 + DS, :])
            for r in range(max(1, NSZ // TC)):
                rs = slice(r * TC, (r + 1) * TC)
                nc.sync.dma_start(
                    bc_scr[j, 0, n * (NSZ // TC) + r]
                    .rearrange("(s t) -> s t", s=DS), bt[:, rs])
                nc.sync.dma_start(
                    bc_scr[j, 1, n * (NSZ // TC) + r]
                    .rearrange("(s t) -> s t", s=DS), ct[:, rs])
        # dt = softplus(psm + b_dt) as Ln(1 + Exp(.)): Exp lands in the dt
        # tile, then one in-place Ln per d-block (Exp and Ln share the
        # natural_log_exp act table; batching avoids per-n table switches).
        dtf = []
        for db in range(ND):
            d_ = dtp.tile([P, L], BF, tag=f"dt{db}")
            for n in range(NN):
                ns = slice(n * NSZ, (n + 1) * NSZ)
                psm = pp.tile([P, NSZ], FP32, tag="ps")
                nc.tensor.matmul(psm[:], wdt_full[:, db * P:(db + 1) * P],
                                 dtl[:, ns], start=True, stop=True)
                nc.scalar.activation(d_[:, ns], psm[:], AF.Exp, bias=bdt3[db])
            nc.scalar.activation(d_[:], d_[:], AF.Ln, bias=1.0)
            dtf.append(d_)

        # ================= D: selective scan =================
        # Scan block layout per s-segment: [TC data][reset col][hold col].
        # reset col: a=0, b=carry(next seg) -> state := carry.
        # hold col:  a=1, b=0               -> state preserved.
        # Segment 0's carry enters via the scan's `initial` operand.
        # Carries kept fp32: bf16->fp32 copies take the fast CAST path.
        hcarry = {}
        for db in range(ND):
            for sb in range(NSB):
                t_ = sm.tile([P, SB, 1], FP32, name=f"hcr{db}{sb}",
                             tag=f"hcr{db}_{sb}", bufs=1)
                hcarry[(db, sb)] = t_
        for ch in range(NCH):
            cs = slice(ch * TC, (ch + 1) * TC)
            uchs = []
            for db in range(ND):
                u_ = sm.tile([P, TC], BF, tag=f"uch{db}", bufs=2)
                nc.vector.tensor_tensor(u_[:], dtf[db][:, cs], xs[db][:, cs],
                                        ALU.mult)
                uchs.append(u_)
            ysum = [[] for _ in range(ND)]
            for sb in range(NSB):
                brep = bcp.tile([P, SB, TC], BF, tag="brep")
                crep = bcp.tile([P, SB, TC], BF, tag="crep")
                nc.sync.dma_start(
                    brep[:],
                    bc_scr[j, 0, ch, sb * SB * TC:(sb + 1) * SB * TC]
                    .rearrange("(s t) -> s t", s=SB)
                    .unsqueeze(0).broadcast_to((P, SB, TC)))
                nc.sync.dma_start(
                    crep[:],
                    bc_scr[j, 1, ch, sb * SB * TC:(sb + 1) * SB * TC]
                    .rearrange("(s t) -> s t", s=SB)
                    .unsqueeze(0).broadcast_to((P, SB, TC)))
                for db in range(ND):
                    a_ = af32.tile([P, SB, TB], FP32, tag="a")
                    for s8 in range(SB):
                        s = sb * SB + s8
                        nc.scalar.activation(a_[:, s8, 0:TC], dtf[db][:, cs],
                                             AF.Exp, scale=ac3[db][:, s:s + 1])
                    nc.vector.memset(a_[:, :, TC:TC + 1], 0.0)
                    nc.vector.memset(a_[:, :, TC + 1:TB], 1.0)
                    b_ = bcls.tile([P, SB, TB], BF, tag="b")
                    uv = uchs[db][:].unsqueeze(1).broadcast_to((P, SB, TC))
                    beng = nc.gpsimd if GP_B else nc.vector
                    beng.tensor_tensor(b_[:, :, 0:TC], uv, brep[:],
                                       ALU.mult)
                    nc.vector.memset(b_[:, :, TC + 1:TB], 0.0)
                    if ch == 0:
                        nc.vector.memset(b_[:, :, TC:TC + 1], 0.0)
                        init = 0.0
                    else:
                        nc.vector.tensor_copy(b_[:, 0:SB - 1, TC:TC + 1],
                                              hcarry[(db, sb)][:, 1:SB, :])
                        nc.vector.memset(b_[:, SB - 1:SB, TC:TC + 1], 0.0)
                        init = hcarry[(db, sb)][:, 0:1, :]
                    h_ = hbf.tile([P, SB, TB], BF, tag="h")
                    seng = nc.gpsimd if sb in GP_SBS else nc.vector
                    seng.tensor_tensor_scan(
                        h_[:].rearrange("p s t -> p (s t)"),
                        a_[:].rearrange("p s t -> p (s t)"),
                        b_[:].rearrange("p s t -> p (s t)"),
                        init, ALU.mult, ALU.add)
                    if ch < NCH - 1:
                        nc.vector.tensor_copy(hcarry[(db, sb)][:],
                                              h_[:, :, TC - 1:TC])
                    heng = nc.gpsimd if GP_HCM else nc.vector
                    hcm = bcls.tile([P, SB, TC], BF, tag="hcm")
                    heng.tensor_tensor(hcm[:], h_[:, :, 0:TC], crep[:],
                                       ALU.mult)
                    t2 = sm.tile([P, 2, TC], BF, tag="t2", bufs=2)
                    heng.tensor_tensor(t2[:], hcm[:, 0:2, :],
                                       hcm[:, 2:4, :], ALU.add)
                    ysb = sm.tile([P, TC], BF, tag=f"ysb{db}", bufs=2)
                    nc.vector.tensor_tensor(ysb[:], t2[:, 0, :],
                                            t2[:, 1, :], ALU.add)
                    ysum[db].append(ysb)
                    if sb == 1:
                        yA = sm.tile([P, TC], BF, tag=f"yA{db}", bufs=1)
                        nc.vector.tensor_tensor(yA[:], ysum[db][0][:],
                                                ysum[db][1][:], ALU.add)
                        ysum[db] = [yA]
            for db in range(ND):
                yB = sm.tile([P, TC], BF, tag="yB", bufs=2)
                nc.vector.tensor_tensor(yB[:], ysum[db][1][:],
                                        ysum[db][2][:], ALU.add)
                xsd = sm.tile([P, TC], BF, tag="xsd", bufs=2)
                nc.vector.tensor_scalar_mul(xsd[:], xs[db][:, cs],
                                            dsk3[db][:])
                ysd = sm.tile([P, TC], BF, tag="ysd", bufs=2)
                nc.vector.tensor_tensor(ysd[:], ysum[db][0][:], yB[:],
                                        ALU.add)
                yf = sm.tile([P, TC], BF, tag="yf", bufs=2)
                nc.vector.tensor_tensor(yf[:], ysd[:], xsd[:], ALU.add)
                nc.sync.dma_start(y_scr[j, db, :, cs], yf[:])

    # ================= E: gate + out-proj =================
    for j in range(NB):
        for n in range(NN):
            ns = slice(n * NSZ, (n + 1) * NSZ)
            ygs = []
            for db in range(ND):
                zt = smE.tile([P, NSZ], BF, tag="ze", bufs=2)
                nc.sync.dma_start(zt[:], z_scr[j, db, :, ns])
                yt = smE.tile([P, NSZ], BF, tag="ye", bufs=2)
                nc.sync.dma_start(yt[:], y_scr[j, db, :, ns])
                nc.vector.tensor_tensor(yt[:], yt[:], zt[:], ALU.mult)
                ygs.append(yt)
            for m in range(2):
                msz = P if m == 0 else C_ - P
                mm = slice(m * P, m * P + msz)
                pso = pp2.tile([msz, NSZ], FP32, tag="pso")
                for db in range(ND):
                    nc.tensor.matmul(pso[:], wo3[db][:, mm], ygs[db][:],
                                     start=(db == 0), stop=(db == ND - 1))
                ot = smE.tile([msz, NSZ], FP32, tag="oe", bufs=1)
                nc.scalar.copy(ot[:], pso[:])
                nc.sync.dma_start(outT[j, mm, ns], ot[:])


# ---------------- host side ----------------

_CACHE = {}
PROFILE = False
PROFILE_KW = {}


def _get_nc():
    if "nc" not in _CACHE:
        _CACHE["nc"] = build_nc()
    return _CACHE["nc"]


def _permute_toks(x, idx):
    """x: [C, H, W] fp32 -> 4 direction token maps, each [C, L] (c-major)."""
    c = x.shape[0]
    row = x.reshape(c, -1)
    col = x.transpose(0, 2, 1).reshape(c, -1)
    diag = row[:, idx]
    anti = x[:, :, ::-1].reshape(c, -1)[:, idx]
    return [row, col, diag, anti]


def _unpermute(outs, inv_idx, h, w):
    """outs: list of 4 [C, L] -> sum of un-permuted direction outputs."""
    c = outs[0].shape[0]
    row_f = outs[0].reshape(c, h, w)
    col_f = outs[1].reshape(c, w, h).transpose(0, 2, 1)
    diag_f = outs[2][:, inv_idx].reshape(c, h, w)
    anti_f = outs[3][:, inv_idx].reshape(c, h, w)[:, :, ::-1]
    return row_f + col_f + diag_f + anti_f


def _pack_convd(cw):
    """Per d-block, per tap: diag(conv_w[:, k]) as bf16 PE weights."""
    out = np.zeros((ND, DC, 128, 128), np.float32)
    for db in range(ND):
        for k in range(DC):
            np.fill_diagonal(out[db, k], cw[db * 128:(db + 1) * 128, k])
    return out.astype(BF16)


def _pack_wx(wx):
    """Pad W_x columns so dt/B/C rows land at PSUM partitions 0/32/64."""
    out = np.zeros((DI, 96), np.float32)
    out[:, 0:DTR] = wx[:, 0:DTR]
    out[:, 32:32 + DS] = wx[:, DTR:DTR + DS]
    out[:, 64:64 + DS] = wx[:, DTR + DS:]
    return out.astype(BF16)


def kernel(x, W_in, conv_w, conv_b, W_x, W_dt, b_dt, A_log, D_skip, W_out,
           idx, inv_idx):
    x = np.asarray(x, np.float32)
    idx = np.asarray(idx, np.int32)
    inv_idx = np.asarray(inv_idx, np.int32)
    A = -np.exp(np.asarray(A_log, np.float32))        # [4, DI, DS]
    conv_b = np.asarray(conv_b, np.float32)
    b_dt = np.asarray(b_dt, np.float32)
    D_skip = np.asarray(D_skip, np.float32)

    nc = _get_nc()
    in_maps = []
    for core in range(N_CORES):
        d = core // 2      # direction
        bh = core % 2      # batch half
        toks = np.empty((NB, C_, L), BF16)
        for jb in range(NB):
            b = bh * NB + jb
            toks[jb] = _permute_toks(x[b], idx)[d].astype(BF16)
        in_maps.append(dict(
            tokT=toks,
            Win=np.asarray(W_in[d], np.float32).astype(BF16),
            convd=_pack_convd(np.asarray(conv_w[d], np.float32)),
            convb=np.ascontiguousarray(conv_b[d].reshape(DI, 1)),
            Wx=_pack_wx(np.asarray(W_x[d], np.float32)),
            Wdt=np.asarray(W_dt[d], np.float32).astype(BF16),
            bdt=np.ascontiguousarray(b_dt[d].reshape(DI, 1)),
            Acoef=np.ascontiguousarray(A[d]),
            Dsk=np.ascontiguousarray(D_skip[d].reshape(DI, 1)),
            Wout=np.asarray(W_out[d], np.float32).astype(BF16),
        ))

    res = run_bass_kernel_spmd(nc, in_maps, list(range(N_CORES)),
                               trace=PROFILE, **PROFILE_KW)
    _CACHE["last_exec_ns"] = res.exec_time_ns
    outs = res.results

    # gather: per batch b, the 4 direction outputs live on cores d*2 + b//2
    acc = np.zeros((B_, C_, H_, W_), np.float32)
    for b in range(B_):
        bh, jb = b // NB, b % NB
        douts = [np.asarray(outs[d * 2 + bh]["outT"][jb], np.float32)
                 for d in range(4)]
        acc[b] = _unpermute(douts, inv_idx, H_, W_)
    gate = 1.0 / (1.0 + np.exp(-0.25 * acc))
    return x * gate


# revision 20
# speedup vs baseline: 1.0821x; 1.0821x over previous
"""AxialCrossMamba Trainium2 kernel.

Sharding: 8 cores = 4 directions x 2 batch-halves. Each core runs one
direction's Mamba block (its own weights) over two batches. Host does the
direction permutations (row/col/diag/anti, c-major [C, L] token layouts),
and the final 4-direction sigmoid gate.

Device pipeline per (job = one batch):
  in-proj matmul (bf16 PE) -> causal depthwise conv (PE) + fused silu (ACT)
  -> x-proj/dt matmuls (PE) -> fused softplus (ACT)
  -> selective scan: a = exp(dt*A) per-state-column ACT activations (fp32),
     b = u*B (bf16), tensor_tensor_scan over flattened (s,t) with
     trailing reset/hold boundary columns (even-width blocks keep the
     DVE 2x perf mode); scans split between Vector and GpSimd engines;
     h*C + tree reduce over s on Vector
  -> y = ys + xs*D (bf16), gate silu(z), out-proj matmul (bf16 PE).
"""

import sys

for _p in ("/opt/trn_rl_repo", "/root/.axon_site/_ro/trn_rl_repo"):
    if _p not in sys.path:
        sys.path.insert(0, _p)

from contextlib import ExitStack

import numpy as np
import ml_dtypes

import concourse.bass as bass
from concourse import bacc
import concourse.mybir as mybir
import concourse.tile as tile
from concourse.bass_utils import run_bass_kernel_spmd

BF16 = ml_dtypes.bfloat16

# Problem constants
B_, C_, H_, W_ = 4, 192, 64, 64
L = H_ * W_          # 4096 tokens
DS, DC = 16, 4       # d_state, d_conv
DI = 2 * C_          # 384 d_inner
DTR = (C_ + 15) // 16  # 12 dt_rank
NB = 2               # batches per core
ND = DI // 128       # 3 d-blocks
N_CORES = 8

GP_SBS = ()          # scans: Vector only (scan opcode invalid on Pool/GpSimd)
GP_B = False         # b = u*B on Vector (1x rate is cheap there)
GP_HCM_SBS = (0,)    # s-blocks whose h*C runs on Pool (rest on Vector)
# The reduce tail (t2/ysb/y-tree) is consumer-only: nothing in phase D
# depends on it, so it runs on the Pool engine and may lag freely.

AF = mybir.ActivationFunctionType
ALU = mybir.AluOpType
FP32 = mybir.dt.float32
BF = mybir.dt.bfloat16


def build_nc(L=L, TC=512, SB=4):
    """Build the SPMD single-core program (identical on all 8 cores)."""
    nc = bacc.Bacc("TRN2", debug=False)

    # ---- DRAM I/O ----
    tokT = nc.dram_tensor("tokT", [NB, C_, L], BF, kind="ExternalInput").ap()
    Win = nc.dram_tensor("Win", [C_, 2 * DI], BF, kind="ExternalInput").ap()
    convd = nc.dram_tensor("convd", [ND, DC, 128, 128], BF, kind="ExternalInput").ap()
    convb = nc.dram_tensor("convb", [DI, 1], FP32, kind="ExternalInput").ap()
    Wx = nc.dram_tensor("Wx", [DI, 96], BF, kind="ExternalInput").ap()
    Wdt = nc.dram_tensor("Wdt", [DTR, DI], BF, kind="ExternalInput").ap()
    bdt = nc.dram_tensor("bdt", [DI, 1], FP32, kind="ExternalInput").ap()
    Acoef = nc.dram_tensor("Acoef", [DI, DS], FP32, kind="ExternalInput").ap()
    Dsk = nc.dram_tensor("Dsk", [DI, 1], FP32, kind="ExternalInput").ap()
    Wout = nc.dram_tensor("Wout", [DI, C_], BF, kind="ExternalInput").ap()
    outT = nc.dram_tensor("outT", [NB, C_, L], FP32, kind="ExternalOutput").ap()
    # scratch
    z_scr = nc.dram_tensor("z_scr", [NB, ND, 128, L], BF, kind="Internal").ap()
    y_scr = nc.dram_tensor("y_scr", [NB, ND, 128, L], BF, kind="Internal").ap()
    bc_scr = nc.dram_tensor("bc_scr", [NB, 2, L // TC, DS * TC], BF, kind="Internal").ap()

    io = dict(tokT=tokT, Win=Win, convd=convd, convb=convb, Wx=Wx, Wdt=Wdt,
              bdt=bdt, Acoef=Acoef, Dsk=Dsk, Wout=Wout, outT=outT,
              z_scr=z_scr, y_scr=y_scr, bc_scr=bc_scr)
    with tile.TileContext(nc) as tc:
        with ExitStack() as ctx:
            _emit(ctx, tc, nc, io, L=L, TC=TC, SB=SB)
    nc.compile()
    return nc


def _emit(ctx, tc, nc, io, *, L, TC, SB):
    tokT, Win, convd, convb, Wx, Wdt, bdt = (
        io["tokT"], io["Win"], io["convd"], io["convb"], io["Wx"], io["Wdt"],
        io["bdt"])
    Acoef, Dsk, Wout, outT = io["Acoef"], io["Dsk"], io["Wout"], io["outT"]
    z_scr, y_scr, bc_scr = io["z_scr"], io["y_scr"], io["bc_scr"]

    P = 128
    NCH = L // TC          # t-chunks
    NSB = DS // SB         # s-blocks
    NN = max(1, L // 512)  # matmul n-chunks
    NSZ = L // NN
    TB = TC + 2            # scan block width (data + reset + hold columns)

    # ---- pools ----
    wp = ctx.enter_context(tc.tile_pool(name="weights", bufs=1))
    big = ctx.enter_context(tc.tile_pool(name="big", bufs=4))    # bf16 [128,L]
    af32 = ctx.enter_context(tc.tile_pool(name="af32", bufs=2))  # fp32 scan a
    hbf = ctx.enter_context(tc.tile_pool(name="hbf", bufs=3))    # bf16 scan h
    bcp = ctx.enter_context(tc.tile_pool(name="bcp", bufs=3))    # brep/crep
    bcls = ctx.enter_context(tc.tile_pool(name="bcls", bufs=2))  # b_/hcm
    dtp = ctx.enter_context(tc.tile_pool(name="dtp", bufs=1))    # dt bf16 resident
    xsp = ctx.enter_context(tc.tile_pool(name="xsp", bufs=1))    # xs bf16 resident
    sm = ctx.enter_context(tc.tile_pool(name="small", bufs=2))
    smE = ctx.enter_context(tc.tile_pool(name="smallE", bufs=2))
    pp = ctx.enter_context(tc.tile_pool(name="psum", bufs=2, space="PSUM"))
    pp2 = ctx.enter_context(tc.tile_pool(name="psum2", bufs=2, space="PSUM"))

    # ---- load weights ----
    win0 = wp.tile([P, 2 * DI], BF, tag="win0")
    win1 = wp.tile([C_ - P, 2 * DI], BF, tag="win1")
    nc.sync.dma_start(win0[:], Win[0:P, :])
    nc.sync.dma_start(win1[:], Win[P:C_, :])
    wdt_full = wp.tile([DTR, DI], BF, tag="wdt")
    nc.sync.dma_start(wdt_full[:], Wdt[:])
    wxs, cw3, cb3, bdt3, ac3, dsk3, wo3 = [], [], [], [], [], [], []
    for db in range(ND):
        r = slice(db * P, (db + 1) * P)
        w1 = wp.tile([P, 96], BF, tag=f"wx{db}")
        nc.sync.dma_start(w1[:], Wx[r, :]); wxs.append(w1)
        wconv = []
        for k in range(DC):
            wck = wp.tile([P, P], BF, tag=f"cw{db}_{k}", name=f"cw{db}_{k}")
            nc.sync.dma_start(wck[:], convd[db, k])
            wconv.append(wck)
        cw3.append(wconv)
        w3 = wp.tile([P, 1], FP32, tag=f"cb{db}")
        nc.sync.dma_start(w3[:], convb[r, :]); cb3.append(w3)
        w4 = wp.tile([P, 1], FP32, tag=f"bdt{db}")
        nc.sync.dma_start(w4[:], bdt[r, :]); bdt3.append(w4)
        w5 = wp.tile([P, DS], FP32, tag=f"ac{db}")
        nc.sync.dma_start(w5[:], Acoef[r, :]); ac3.append(w5)
        w6 = wp.tile([P, 1], FP32, tag=f"dsk{db}")
        nc.sync.dma_start(w6[:], Dsk[r, :]); dsk3.append(w6)
        w7 = wp.tile([P, C_], BF, tag=f"wo{db}")
        nc.sync.dma_start(w7[:], Wout[r, :]); wo3.append(w7)

    for j in range(NB):
        # ================= A: in-proj (+ conv interleaved) =================
        tok0 = big.tile([P, L], BF, tag="big")
        tok1 = big.tile([C_ - P, L], BF, tag="big")
        nc.sync.dma_start(tok0[:], tokT[j, 0:P, :])
        nc.sync.dma_start(tok1[:], tokT[j, P:C_, :])

        xs = []
        for m in range(2 * DI // P):   # M-blocks of xz^T; 0..2 -> xi, 3..5 -> z
            if m < ND:
                xi = big.tile([P, L + DC], BF, tag="big")
                nc.scalar.memzero(xi[:, 0:DC])
            mm = slice(m * P, (m + 1) * P)
            for n in range(NN):
                ns = slice(n * NSZ, (n + 1) * NSZ)
                ps = pp.tile([P, NSZ], FP32, tag="ps")
                nc.tensor.matmul(ps[:], win0[:, mm], tok0[:, ns],
                                 start=True, stop=False)
                nc.tensor.matmul(ps[:], win1[:, mm], tok1[:, ns],
                                 start=False, stop=True)
                if m < ND:
                    nc.scalar.copy(xi[:, DC + n * NSZ: DC + (n + 1) * NSZ],
                                   ps[:])
                else:
                    zt = smE.tile([P, NSZ], BF, tag="ztmp", bufs=2)
                    nc.scalar.activation(zt[:], ps[:], AF.Silu)
                    nc.sync.dma_start(z_scr[j, m - ND, :, ns], zt[:])
            if m < ND:
                # conv on PE via diagonal weight matrices, then fused silu
                db = m
                x_ = xsp.tile([P, L], BF, tag=f"xs{db}")
                for n in range(NN):
                    ns = slice(n * NSZ, (n + 1) * NSZ)
                    psc = pp.tile([P, NSZ], FP32, tag="psc")
                    for k in range(DC):
                        nc.tensor.matmul(
                            psc[:], cw3[db][k][:],
                            xi[:, 1 + k + n * NSZ: 1 + k + n * NSZ + NSZ],
                            start=(k == 0), stop=(k == DC - 1))
                    nc.scalar.activation(x_[:, ns], psc[:], AF.Silu,
                                         bias=cb3[db])
                xs.append(x_)

        # ================= C: dbc, dt =================
        dtl = sm.tile([DTR, L], BF, tag="dtl", bufs=1)
        for n in range(NN):
            ns = slice(n * NSZ, (n + 1) * NSZ)
            psd = pp2.tile([96, NSZ], FP32, tag="psd")
            for db in range(ND):
                nc.tensor.matmul(psd[:], wxs[db][:], xs[db][:, ns],
                                 start=(db == 0), stop=(db == ND - 1))
            nc.scalar.copy(dtl[:, ns], psd[0:DTR, :])
            bt = smE.tile([DS, NSZ], BF, tag="bct")
            ct = smE.tile([DS, NSZ], BF, tag="bct")
            nc.scalar.copy(bt[:], psd[32:32 + DS, :])
            nc.scalar.copy(ct[:], psd[64:64 + DS, :])
            for r in range(max(1, NSZ // TC)):
                rs = slice(r * TC, (r + 1) * TC)
                nc.sync.dma_start(
                    bc_scr[j, 0, n * (NSZ // TC) + r]
                    .rearrange("(s t) -> s t", s=DS), bt[:, rs])
                nc.sync.dma_start(
                    bc_scr[j, 1, n * (NSZ // TC) + r]
                    .rearrange("(s t) -> s t", s=DS), ct[:, rs])
        # dt = softplus(psm + b_dt) as Ln(1 + Exp(.)): Exp lands in the dt
        # tile, then one in-place Ln per d-block (Exp and Ln share the
        # natural_log_exp act table; batching avoids per-n table switches).
        dtf = []
        for db in range(ND):
            d_ = dtp.tile([P, L], BF, tag=f"dt{db}")
            for n in range(NN):
                ns = slice(n * NSZ, (n + 1) * NSZ)
                psm = pp.tile([P, NSZ], FP32, tag="ps")
                nc.tensor.matmul(psm[:], wdt_full[:, db * P:(db + 1) * P],
                                 dtl[:, ns], start=True, stop=True)
                nc.scalar.activation(d_[:, ns], psm[:], AF.Exp, bias=bdt3[db])
            nc.scalar.activation(d_[:], d_[:], AF.Ln, bias=1.0)
            dtf.append(d_)

        # ================= D: selective scan =================
        # Scan block layout per s-segment: [TC data][reset col][hold col].
        # reset col: a=0, b=carry(next seg) -> state := carry.
        # hold col:  a=1, b=0               -> state preserved.
        # Segment 0's carry enters via the scan's `initial` operand.
        # Carries kept fp32: bf16->fp32 copies take the fast CAST path.
        hcarry = {}
        for db in range(ND):
            for sb in range(NSB):
                t_ = sm.tile([P, SB, 1], FP32, name=f"hcr{db}{sb}",
                             tag=f"hcr{db}_{sb}", bufs=1)
                hcarry[(db, sb)] = t_
        for ch in range(NCH):
            cs = slice(ch * TC, (ch + 1) * TC)
            uchs = []
            for db in range(ND):
                u_ = sm.tile([P, TC], BF, tag=f"uch{db}", bufs=2)
                nc.vector.tensor_tensor(u_[:], dtf[db][:, cs], xs[db][:, cs],
                                        ALU.mult)
                uchs.append(u_)
            ysum = [[] for _ in range(ND)]
            for sb in range(NSB):
                brep = bcp.tile([P, SB, TC], BF, tag="brep")
                crep = bcp.tile([P, SB, TC], BF, tag="crep")
                nc.sync.dma_start(
                    brep[:],
                    bc_scr[j, 0, ch, sb * SB * TC:(sb + 1) * SB * TC]
                    .rearrange("(s t) -> s t", s=SB)
                    .unsqueeze(0).broadcast_to((P, SB, TC)))
                nc.sync.dma_start(
                    crep[:],
                    bc_scr[j, 1, ch, sb * SB * TC:(sb + 1) * SB * TC]
                    .rearrange("(s t) -> s t", s=SB)
                    .unsqueeze(0).broadcast_to((P, SB, TC)))
                for db in range(ND):
                    a_ = af32.tile([P, SB, TB], FP32, tag="a")
                    for s8 in range(SB):
                        s = sb * SB + s8
                        nc.scalar.activation(a_[:, s8, 0:TC], dtf[db][:, cs],
                                             AF.Exp, scale=ac3[db][:, s:s + 1])
                    nc.vector.memset(a_[:, :, TC:TC + 1], 0.0)
                    nc.vector.memset(a_[:, :, TC + 1:TB], 1.0)
                    b_ = bcls.tile([P, SB, TB], BF, tag="b")
                    uv = uchs[db][:].unsqueeze(1).broadcast_to((P, SB, TC))
                    beng = nc.gpsimd if GP_B else nc.vector
                    beng.tensor_tensor(b_[:, :, 0:TC], uv, brep[:],
                                       ALU.mult)
                    nc.vector.memset(b_[:, :, TC + 1:TB], 0.0)
                    if ch == 0:
                        nc.vector.memset(b_[:, :, TC:TC + 1], 0.0)
                        init = 0.0
                    else:
                        nc.vector.tensor_copy(b_[:, 0:SB - 1, TC:TC + 1],
                                              hcarry[(db, sb)][:, 1:SB, :])
                        nc.vector.memset(b_[:, SB - 1:SB, TC:TC + 1], 0.0)
                        init = hcarry[(db, sb)][:, 0:1, :]
                    h_ = hbf.tile([P, SB, TB], BF, tag="h")
                    seng = nc.gpsimd if sb in GP_SBS else nc.vector
                    seng.tensor_tensor_scan(
                        h_[:].rearrange("p s t -> p (s t)"),
                        a_[:].rearrange("p s t -> p (s t)"),
                        b_[:].rearrange("p s t -> p (s t)"),
                        init, ALU.mult, ALU.add)
                    if ch < NCH - 1:
                        nc.vector.tensor_copy(hcarry[(db, sb)][:],
                                              h_[:, :, TC - 1:TC])
                    heng = nc.gpsimd if sb in GP_HCM_SBS else nc.vector
                    hcm = bcls.tile([P, SB, TC], BF, tag="hcm")
                    heng.tensor_tensor(hcm[:], h_[:, :, 0:TC], crep[:],
                                       ALU.mult)
                    t2 = sm.tile([P, 2, TC], BF, tag="t2", bufs=2)
                    nc.gpsimd.tensor_tensor(t2[:], hcm[:, 0:2, :],
                                            hcm[:, 2:4, :], ALU.add)
                    ysb = sm.tile([P, TC], BF, tag=f"ysb{db}", bufs=2)
                    nc.gpsimd.tensor_tensor(ysb[:], t2[:, 0, :],
                                            t2[:, 1, :], ALU.add)
                    ysum[db].append(ysb)
                    if sb == 1:
                        yA = sm.tile([P, TC], BF, tag=f"yA{db}", bufs=1)
                        nc.gpsimd.tensor_tensor(yA[:], ysum[db][0][:],
                                                ysum[db][1][:], ALU.add)
                        ysum[db] = [yA]
            for db in range(ND):
                yB = sm.tile([P, TC], BF, tag="yB", bufs=2)
                nc.gpsimd.tensor_tensor(yB[:], ysum[db][1][:],
                                        ysum[db][2][:], ALU.add)
                xsd = sm.tile([P, TC], BF, tag="xsd", bufs=2)
                nc.gpsimd.tensor_tensor(
                    xsd[:], xs[db][:, cs],
                    dsk3[db][:].broadcast_to((P, TC)), ALU.mult)
                ysd = sm.tile([P, TC], BF, tag="ysd", bufs=2)
                nc.gpsimd.tensor_tensor(ysd[:], ysum[db][0][:], yB[:],
                                        ALU.add)
                yf = sm.tile([P, TC], BF, tag="yf", bufs=2)
                nc.gpsimd.tensor_tensor(yf[:], ysd[:], xsd[:], ALU.add)
                nc.sync.dma_start(y_scr[j, db, :, cs], yf[:])

    # ================= E: gate + out-proj =================
    for j in range(NB):
        for n in range(NN):
            ns = slice(n * NSZ, (n + 1) * NSZ)
            ygs = []
            for db in range(ND):
                zt = smE.tile([P, NSZ], BF, tag="ze", bufs=2)
                nc.sync.dma_start(zt[:], z_scr[j, db, :, ns])
                yt = smE.tile([P, NSZ], BF, tag="ye", bufs=2)
                nc.sync.dma_start(yt[:], y_scr[j, db, :, ns])
                nc.gpsimd.tensor_tensor(yt[:], yt[:], zt[:], ALU.mult)
                ygs.append(yt)
            for m in range(2):
                msz = P if m == 0 else C_ - P
                mm = slice(m * P, m * P + msz)
                pso = pp2.tile([msz, NSZ], FP32, tag="pso")
                for db in range(ND):
                    nc.tensor.matmul(pso[:], wo3[db][:, mm], ygs[db][:],
                                     start=(db == 0), stop=(db == ND - 1))
                ot = smE.tile([msz, NSZ], FP32, tag="oe", bufs=1)
                nc.scalar.copy(ot[:], pso[:])
                nc.sync.dma_start(outT[j, mm, ns], ot[:])


# ---------------- host side ----------------

_CACHE = {}
PROFILE = False
PROFILE_KW = {}


def _get_nc():
    if "nc" not in _CACHE:
        _CACHE["nc"] = build_nc()
    return _CACHE["nc"]


def _permute_toks(x, idx):
    """x: [C, H, W] fp32 -> 4 direction token maps, each [C, L] (c-major)."""
    c = x.shape[0]
    row = x.reshape(c, -1)
    col = x.transpose(0, 2, 1).reshape(c, -1)
    diag = row[:, idx]
    anti = x[:, :, ::-1].reshape(c, -1)[:, idx]
    return [row, col, diag, anti]


def _unpermute(outs, inv_idx, h, w):
    """outs: list of 4 [C, L] -> sum of un-permuted direction outputs."""
    c = outs[0].shape[0]
    row_f = outs[0].reshape(c, h, w)
    col_f = outs[1].reshape(c, w, h).transpose(0, 2, 1)
    diag_f = outs[2][:, inv_idx].reshape(c, h, w)
    anti_f = outs[3][:, inv_idx].reshape(c, h, w)[:, :, ::-1]
    return row_f + col_f + diag_f + anti_f


def _pack_convd(cw):
    """Per d-block, per tap: diag(conv_w[:, k]) as bf16 PE weights."""
    out = np.zeros((ND, DC, 128, 128), np.float32)
    for db in range(ND):
        for k in range(DC):
            np.fill_diagonal(out[db, k], cw[db * 128:(db + 1) * 128, k])
    return out.astype(BF16)


def _pack_wx(wx):
    """Pad W_x columns so dt/B/C rows land at PSUM partitions 0/32/64."""
    out = np.zeros((DI, 96), np.float32)
    out[:, 0:DTR] = wx[:, 0:DTR]
    out[:, 32:32 + DS] = wx[:, DTR:DTR + DS]
    out[:, 64:64 + DS] = wx[:, DTR + DS:]
    return out.astype(BF16)


def kernel(x, W_in, conv_w, conv_b, W_x, W_dt, b_dt, A_log, D_skip, W_out,
           idx, inv_idx):
    x = np.asarray(x, np.float32)
    idx = np.asarray(idx, np.int32)
    inv_idx = np.asarray(inv_idx, np.int32)
    A = -np.exp(np.asarray(A_log, np.float32))        # [4, DI, DS]
    conv_b = np.asarray(conv_b, np.float32)
    b_dt = np.asarray(b_dt, np.float32)
    D_skip = np.asarray(D_skip, np.float32)

    nc = _get_nc()
    in_maps = []
    for core in range(N_CORES):
        d = core // 2      # direction
        bh = core % 2      # batch half
        toks = np.empty((NB, C_, L), BF16)
        for jb in range(NB):
            b = bh * NB + jb
            toks[jb] = _permute_toks(x[b], idx)[d].astype(BF16)
        in_maps.append(dict(
            tokT=toks,
            Win=np.asarray(W_in[d], np.float32).astype(BF16),
            convd=_pack_convd(np.asarray(conv_w[d], np.float32)),
            convb=np.ascontiguousarray(conv_b[d].reshape(DI, 1)),
            Wx=_pack_wx(np.asarray(W_x[d], np.float32)),
            Wdt=np.asarray(W_dt[d], np.float32).astype(BF16),
            bdt=np.ascontiguousarray(b_dt[d].reshape(DI, 1)),
            Acoef=np.ascontiguousarray(A[d]),
            Dsk=np.ascontiguousarray(D_skip[d].reshape(DI, 1)),
            Wout=np.asarray(W_out[d], np.float32).astype(BF16),
        ))

    res = run_bass_kernel_spmd(nc, in_maps, list(range(N_CORES)),
                               trace=PROFILE, **PROFILE_KW)
    _CACHE["last_exec_ns"] = res.exec_time_ns
    outs = res.results

    # gather: per batch b, the 4 direction outputs live on cores d*2 + b//2
    acc = np.zeros((B_, C_, H_, W_), np.float32)
    for b in range(B_):
        bh, jb = b // NB, b % NB
        douts = [np.asarray(outs[d * 2 + bh]["outT"][jb], np.float32)
                 for d in range(4)]
        acc[b] = _unpermute(douts, inv_idx, H_, W_)
    gate = 1.0 / (1.0 + np.exp(-0.25 * acc))
    return x * gate


# revision 27
# speedup vs baseline: 1.3725x; 1.2683x over previous
"""AxialCrossMamba Trainium2 kernel.

Sharding: 8 cores = 4 directions x 2 batch-halves. Each core runs one
direction's Mamba block (its own weights) over two batches. Host does the
direction permutations (row/col/diag/anti, c-major [C, L] token layouts),
and the final 4-direction sigmoid gate.

Device pipeline per (job = one batch):
  in-proj matmul (bf16 PE) -> causal depthwise conv (PE) + fused silu (ACT)
  -> x-proj/dt matmuls (PE) -> fused softplus (ACT)
  -> selective scan: a = exp(dt*A) per-state-column ACT activations (fp32),
     b = u*B (bf16), tensor_tensor_scan over flattened (s,t) with
     trailing reset/hold boundary columns (even-width blocks keep the
     DVE 2x perf mode); scans split between Vector and GpSimd engines;
     h*C + tree reduce over s on Vector
  -> y = ys + xs*D (bf16), gate silu(z), out-proj matmul (bf16 PE).
"""

import sys

for _p in ("/opt/trn_rl_repo", "/root/.axon_site/_ro/trn_rl_repo"):
    if _p not in sys.path:
        sys.path.insert(0, _p)

from contextlib import ExitStack

import numpy as np
import ml_dtypes

import concourse.bass as bass
from concourse import bacc
import concourse.mybir as mybir
import concourse.tile as tile
from concourse.bass_utils import run_bass_kernel_spmd

BF16 = ml_dtypes.bfloat16

# Problem constants
B_, C_, H_, W_ = 4, 192, 64, 64
L = H_ * W_          # 4096 tokens
DS, DC = 16, 4       # d_state, d_conv
DI = 2 * C_          # 384 d_inner
DTR = (C_ + 15) // 16  # 12 dt_rank
NB = 2               # batches per core
ND = DI // 128       # 3 d-blocks
N_CORES = 8

# All elementwise work stays on Vector: the Pool/GpSimd engine runs these
# ops ~8x slower and its SBUF traffic inflates concurrent Vector ops
# 25-40% (measured), so offloading is a net loss. The scan opcode is
# invalid on Pool anyway.

AF = mybir.ActivationFunctionType
ALU = mybir.AluOpType
FP32 = mybir.dt.float32
BF = mybir.dt.bfloat16


def build_nc(L=L, TC=512, SB=4):
    """Build the SPMD single-core program (identical on all 8 cores)."""
    nc = bacc.Bacc("TRN2", debug=False)

    # ---- DRAM I/O ----
    tokT = nc.dram_tensor("tokT", [NB, C_, L], BF, kind="ExternalInput").ap()
    Win = nc.dram_tensor("Win", [C_, 2 * DI], BF, kind="ExternalInput").ap()
    convd = nc.dram_tensor("convd", [ND, DC, 128, 128], BF, kind="ExternalInput").ap()
    convb = nc.dram_tensor("convb", [DI, 1], FP32, kind="ExternalInput").ap()
    Wx = nc.dram_tensor("Wx", [DI, 96], BF, kind="ExternalInput").ap()
    Wdt = nc.dram_tensor("Wdt", [DTR, DI], BF, kind="ExternalInput").ap()
    bdt = nc.dram_tensor("bdt", [DI, 1], FP32, kind="ExternalInput").ap()
    Acoef = nc.dram_tensor("Acoef", [DI, DS], FP32, kind="ExternalInput").ap()
    Dsk = nc.dram_tensor("Dsk", [DI, 1], FP32, kind="ExternalInput").ap()
    Wout = nc.dram_tensor("Wout", [DI, C_], BF, kind="ExternalInput").ap()
    outT = nc.dram_tensor("outT", [NB, C_, L], FP32, kind="ExternalOutput").ap()
    # scratch
    z_scr = nc.dram_tensor("z_scr", [NB, ND, 128, L], BF, kind="Internal").ap()
    y_scr = nc.dram_tensor("y_scr", [NB, ND, 128, L], BF, kind="Internal").ap()
    bc_scr = nc.dram_tensor("bc_scr", [NB, 2, L // TC, DS * TC], BF, kind="Internal").ap()

    io = dict(tokT=tokT, Win=Win, convd=convd, convb=convb, Wx=Wx, Wdt=Wdt,
              bdt=bdt, Acoef=Acoef, Dsk=Dsk, Wout=Wout, outT=outT,
              z_scr=z_scr, y_scr=y_scr, bc_scr=bc_scr)
    with tile.TileContext(nc) as tc:
        with ExitStack() as ctx:
            _emit(ctx, tc, nc, io, L=L, TC=TC, SB=SB)
    nc.compile()
    return nc


def _emit(ctx, tc, nc, io, *, L, TC, SB):
    tokT, Win, convd, convb, Wx, Wdt, bdt = (
        io["tokT"], io["Win"], io["convd"], io["convb"], io["Wx"], io["Wdt"],
        io["bdt"])
    Acoef, Dsk, Wout, outT = io["Acoef"], io["Dsk"], io["Wout"], io["outT"]
    z_scr, y_scr, bc_scr = io["z_scr"], io["y_scr"], io["bc_scr"]

    P = 128
    NCH = L // TC          # t-chunks
    NSB = DS // SB         # s-blocks
    NN = max(1, L // 512)  # matmul n-chunks
    NSZ = L // NN
    TB = TC + 2            # scan block width (data + reset + hold columns)

    # ---- pools ----
    wp = ctx.enter_context(tc.tile_pool(name="weights", bufs=1))
    big = ctx.enter_context(tc.tile_pool(name="big", bufs=4))    # bf16 [128,L]
    af32 = ctx.enter_context(tc.tile_pool(name="af32", bufs=2))  # fp32 scan a
    hbf = ctx.enter_context(tc.tile_pool(name="hbf", bufs=3))    # bf16 scan h
    bcp = ctx.enter_context(tc.tile_pool(name="bcp", bufs=3))    # brep/crep
    bcls = ctx.enter_context(tc.tile_pool(name="bcls", bufs=2))  # b_/hcm
    dtp = ctx.enter_context(tc.tile_pool(name="dtp", bufs=1))    # dt bf16 resident
    xsp = ctx.enter_context(tc.tile_pool(name="xsp", bufs=1))    # xs bf16 resident
    sm = ctx.enter_context(tc.tile_pool(name="small", bufs=2))
    smE = ctx.enter_context(tc.tile_pool(name="smallE", bufs=2))
    pp = ctx.enter_context(tc.tile_pool(name="psum", bufs=2, space="PSUM"))
    pp2 = ctx.enter_context(tc.tile_pool(name="psum2", bufs=2, space="PSUM"))

    # ---- load weights ----
    win0 = wp.tile([P, 2 * DI], BF, tag="win0")
    win1 = wp.tile([C_ - P, 2 * DI], BF, tag="win1")
    nc.sync.dma_start(win0[:], Win[0:P, :])
    nc.sync.dma_start(win1[:], Win[P:C_, :])
    wdt_full = wp.tile([DTR, DI], BF, tag="wdt")
    nc.sync.dma_start(wdt_full[:], Wdt[:])
    wxs, cw3, cb3, bdt3, ac3, dsk3, wo3 = [], [], [], [], [], [], []
    for db in range(ND):
        r = slice(db * P, (db + 1) * P)
        w1 = wp.tile([P, 96], BF, tag=f"wx{db}")
        nc.sync.dma_start(w1[:], Wx[r, :]); wxs.append(w1)
        wconv = []
        for k in range(DC):
            wck = wp.tile([P, P], BF, tag=f"cw{db}_{k}", name=f"cw{db}_{k}")
            nc.sync.dma_start(wck[:], convd[db, k])
            wconv.append(wck)
        cw3.append(wconv)
        w3 = wp.tile([P, 1], FP32, tag=f"cb{db}")
        nc.sync.dma_start(w3[:], convb[r, :]); cb3.append(w3)
        w4 = wp.tile([P, 1], FP32, tag=f"bdt{db}")
        nc.sync.dma_start(w4[:], bdt[r, :]); bdt3.append(w4)
        w5 = wp.tile([P, DS], FP32, tag=f"ac{db}")
        nc.sync.dma_start(w5[:], Acoef[r, :]); ac3.append(w5)
        w6 = wp.tile([P, 1], FP32, tag=f"dsk{db}")
        nc.sync.dma_start(w6[:], Dsk[r, :]); dsk3.append(w6)
        w7 = wp.tile([P, C_], BF, tag=f"wo{db}")
        nc.sync.dma_start(w7[:], Wout[r, :]); wo3.append(w7)

    for j in range(NB):
        # ================= A: in-proj (+ conv interleaved) =================
        tok0 = big.tile([P, L], BF, tag="big")
        tok1 = big.tile([C_ - P, L], BF, tag="big")
        nc.sync.dma_start(tok0[:], tokT[j, 0:P, :])
        nc.sync.dma_start(tok1[:], tokT[j, P:C_, :])

        xs = []
        for m in range(2 * DI // P):   # M-blocks of xz^T; 0..2 -> xi, 3..5 -> z
            if m < ND:
                xi = big.tile([P, L + DC], BF, tag="big")
                nc.scalar.memzero(xi[:, 0:DC])
            mm = slice(m * P, (m + 1) * P)
            for n in range(NN):
                ns = slice(n * NSZ, (n + 1) * NSZ)
                ps = pp.tile([P, NSZ], FP32, tag="ps")
                nc.tensor.matmul(ps[:], win0[:, mm], tok0[:, ns],
                                 start=True, stop=False)
                nc.tensor.matmul(ps[:], win1[:, mm], tok1[:, ns],
                                 start=False, stop=True)
                if m < ND:
                    nc.scalar.copy(xi[:, DC + n * NSZ: DC + (n + 1) * NSZ],
                                   ps[:])
                else:
                    zt = smE.tile([P, NSZ], BF, tag="ztmp", bufs=2)
                    nc.scalar.activation(zt[:], ps[:], AF.Silu)
                    nc.sync.dma_start(z_scr[j, m - ND, :, ns], zt[:])
            if m < ND:
                # conv on PE via diagonal weight matrices, then fused silu
                db = m
                x_ = xsp.tile([P, L], BF, tag=f"xs{db}")
                for n in range(NN):
                    ns = slice(n * NSZ, (n + 1) * NSZ)
                    psc = pp.tile([P, NSZ], FP32, tag="psc")
                    for k in range(DC):
                        nc.tensor.matmul(
                            psc[:], cw3[db][k][:],
                            xi[:, 1 + k + n * NSZ: 1 + k + n * NSZ + NSZ],
                            start=(k == 0), stop=(k == DC - 1))
                    nc.scalar.activation(x_[:, ns], psc[:], AF.Silu,
                                         bias=cb3[db])
                xs.append(x_)

        # ================= C: dbc, dt =================
        dtl = sm.tile([DTR, L], BF, tag="dtl", bufs=1)
        for n in range(NN):
            ns = slice(n * NSZ, (n + 1) * NSZ)
            psd = pp2.tile([96, NSZ], FP32, tag="psd")
            for db in range(ND):
                nc.tensor.matmul(psd[:], wxs[db][:], xs[db][:, ns],
                                 start=(db == 0), stop=(db == ND - 1))
            nc.scalar.copy(dtl[:, ns], psd[0:DTR, :])
            bt = smE.tile([DS, NSZ], BF, tag="bct")
            ct = smE.tile([DS, NSZ], BF, tag="bct")
            nc.scalar.copy(bt[:], psd[32:32 + DS, :])
            nc.scalar.copy(ct[:], psd[64:64 + DS, :])
            for r in range(max(1, NSZ // TC)):
                rs = slice(r * TC, (r + 1) * TC)
                nc.sync.dma_start(
                    bc_scr[j, 0, n * (NSZ // TC) + r]
                    .rearrange("(s t) -> s t", s=DS), bt[:, rs])
                nc.sync.dma_start(
                    bc_scr[j, 1, n * (NSZ // TC) + r]
                    .rearrange("(s t) -> s t", s=DS), ct[:, rs])
        # dt = softplus(psm + b_dt) as Ln(1 + Exp(.)): Exp lands in the dt
        # tile, then one in-place Ln per d-block (Exp and Ln share the
        # natural_log_exp act table; batching avoids per-n table switches).
        # dtf is stored per-chunk padded [NCH, TC+2]; the two pad columns
        # become the scan's reset (dt=30 -> a=exp(-30s)=0) and hold
        # (dt=0 -> a=1) columns so the a-exp writes boundary cols for free.
        dtf = []
        for db in range(ND):
            d_ = dtp.tile([P, NCH, TB], BF, tag=f"dt{db}")
            for n in range(NN):
                psm = pp.tile([P, NSZ], FP32, tag="ps")
                nc.tensor.matmul(psm[:], wdt_full[:, db * P:(db + 1) * P],
                                 dtl[:, n * NSZ:(n + 1) * NSZ],
                                 start=True, stop=True)
                nc.scalar.activation(d_[:, n, 0:TC], psm[:], AF.Exp,
                                     bias=bdt3[db])
            nc.vector.memset(d_[:, :, TC:TC + 1], 1e13)  # ln(1+.) -> ~30
            nc.vector.memset(d_[:, :, TC + 1:TB], 0.0)   # ln(1+0) -> 0
            nc.scalar.activation(d_[:], d_[:], AF.Ln, bias=1.0)
            dtf.append(d_)

        # ================= D: selective scan =================
        # Scan block layout per s-segment: [TC data][reset col][hold col].
        # reset col: a=0, b=carry(next seg) -> state := carry.
        # hold col:  a=1, b=0               -> state preserved.
        # Segment 0's carry enters via the scan's `initial` operand.
        # Carries kept fp32: bf16->fp32 copies take the fast CAST path.
        # Slot layout [c0..c3, 0]: slots 1..4 shifted into the reset cols in
        # one copy (seg s resets to carry of seg s+1; the last gets 0).
        hcarry = {}
        for db in range(ND):
            for sb in range(NSB):
                t_ = sm.tile([P, SB + 1, 1], FP32, name=f"hcr{db}{sb}",
                             tag=f"hcr{db}_{sb}", bufs=1)
                nc.vector.memset(t_[:, SB:SB + 1, :], 0.0)
                hcarry[(db, sb)] = t_
        for ch in range(NCH):
            cs = slice(ch * TC, (ch + 1) * TC)
            uchs = []
            for db in range(ND):
                u_ = sm.tile([P, TC], BF, tag=f"uch{db}", bufs=2)
                nc.vector.tensor_tensor(u_[:], dtf[db][:, ch, 0:TC],
                                        xs[db][:, cs], ALU.mult)
                uchs.append(u_)
            ysum = [[] for _ in range(ND)]
            for sb in range(NSB):
                brep = bcp.tile([P, SB, TC], BF, tag="brep")
                crep = bcp.tile([P, SB, TC], BF, tag="crep")
                nc.sync.dma_start(
                    brep[:],
                    bc_scr[j, 0, ch, sb * SB * TC:(sb + 1) * SB * TC]
                    .rearrange("(s t) -> s t", s=SB)
                    .unsqueeze(0).broadcast_to((P, SB, TC)))
                nc.sync.dma_start(
                    crep[:],
                    bc_scr[j, 1, ch, sb * SB * TC:(sb + 1) * SB * TC]
                    .rearrange("(s t) -> s t", s=SB)
                    .unsqueeze(0).broadcast_to((P, SB, TC)))
                for db in range(ND):
                    a_ = af32.tile([P, SB, TB], FP32, tag="a")
                    for s8 in range(SB):
                        s = sb * SB + s8
                        nc.scalar.activation(a_[:, s8, :], dtf[db][:, ch, :],
                                             AF.Exp, scale=ac3[db][:, s:s + 1])
                    b_ = bcls.tile([P, SB, TB], BF, tag="b")
                    uv = uchs[db][:].unsqueeze(1).broadcast_to((P, SB, TC))
                    nc.vector.tensor_tensor(b_[:, :, 0:TC], uv, brep[:],
                                            ALU.mult)
                    nc.vector.memset(b_[:, :, TC + 1:TB], 0.0)
                    if ch == 0:
                        nc.vector.memset(b_[:, :, TC:TC + 1], 0.0)
                        init = 0.0
                    else:
                        nc.vector.tensor_copy(b_[:, :, TC:TC + 1],
                                              hcarry[(db, sb)][:, 1:SB + 1, :])
                        init = hcarry[(db, sb)][:, 0:1, :]
                    h_ = hbf.tile([P, SB, TB], BF, tag="h")
                    nc.vector.tensor_tensor_scan(
                        h_[:].rearrange("p s t -> p (s t)"),
                        a_[:].rearrange("p s t -> p (s t)"),
                        b_[:].rearrange("p s t -> p (s t)"),
                        init, ALU.mult, ALU.add)
                    if ch < NCH - 1:
                        nc.vector.tensor_copy(hcarry[(db, sb)][:, 0:SB, :],
                                              h_[:, :, TC - 1:TC])
                    hcm = bcls.tile([P, SB, TC], BF, tag="hcm")
                    nc.vector.tensor_tensor(hcm[:], h_[:, :, 0:TC], crep[:],
                                            ALU.mult)
                    t2 = sm.tile([P, 2, TC], BF, tag="t2", bufs=2)
                    nc.vector.tensor_tensor(t2[:], hcm[:, 0:2, :],
                                            hcm[:, 2:4, :], ALU.add)
                    ysb = sm.tile([P, TC], BF, tag=f"ysb{db}", bufs=2)
                    nc.vector.tensor_tensor(ysb[:], t2[:, 0, :],
                                            t2[:, 1, :], ALU.add)
                    ysum[db].append(ysb)
                    if sb == 1:
                        yA = sm.tile([P, TC], BF, tag=f"yA{db}", bufs=1)
                        nc.vector.tensor_tensor(yA[:], ysum[db][0][:],
                                                ysum[db][1][:], ALU.add)
                        ysum[db] = [yA]
            for db in range(ND):
                yB = sm.tile([P, TC], BF, tag="yB", bufs=2)
                nc.vector.tensor_tensor(yB[:], ysum[db][1][:],
                                        ysum[db][2][:], ALU.add)
                xsd = sm.tile([P, TC], BF, tag="xsd", bufs=2)
                nc.vector.tensor_scalar_mul(xsd[:], xs[db][:, cs],
                                            dsk3[db][:])
                ysd = sm.tile([P, TC], BF, tag="ysd", bufs=2)
                nc.vector.tensor_tensor(ysd[:], ysum[db][0][:], yB[:],
                                        ALU.add)
                yf = sm.tile([P, TC], BF, tag="yf", bufs=2)
                nc.vector.tensor_tensor(yf[:], ysd[:], xsd[:], ALU.add)
                nc.sync.dma_start(y_scr[j, db, :, cs], yf[:])

    # ================= E: gate + out-proj =================
    for j in range(NB):
        for n in range(NN):
            ns = slice(n * NSZ, (n + 1) * NSZ)
            ygs = []
            for db in range(ND):
                zt = smE.tile([P, NSZ], BF, tag="ze", bufs=2)
                nc.sync.dma_start(zt[:], z_scr[j, db, :, ns])
                yt = smE.tile([P, NSZ], BF, tag="ye", bufs=2)
                nc.sync.dma_start(yt[:], y_scr[j, db, :, ns])
                nc.vector.tensor_tensor(yt[:], yt[:], zt[:], ALU.mult)
                ygs.append(yt)
            for m in range(2):
                msz = P if m == 0 else C_ - P
                mm = slice(m * P, m * P + msz)
                pso = pp2.tile([msz, NSZ], FP32, tag="pso")
                for db in range(ND):
                    nc.tensor.matmul(pso[:], wo3[db][:, mm], ygs[db][:],
                                     start=(db == 0), stop=(db == ND - 1))
                ot = smE.tile([msz, NSZ], FP32, tag="oe", bufs=1)
                nc.scalar.copy(ot[:], pso[:])
                nc.sync.dma_start(outT[j, mm, ns], ot[:])


# ---------------- host side ----------------

_CACHE = {}
PROFILE = False
PROFILE_KW = {}


def _get_nc():
    if "nc" not in _CACHE:
        _CACHE["nc"] = build_nc()
    return _CACHE["nc"]


def _permute_toks(x, idx):
    """x: [C, H, W] fp32 -> 4 direction token maps, each [C, L] (c-major)."""
    c = x.shape[0]
    row = x.reshape(c, -1)
    col = x.transpose(0, 2, 1).reshape(c, -1)
    diag = row[:, idx]
    anti = x[:, :, ::-1].reshape(c, -1)[:, idx]
    return [row, col, diag, anti]


def _unpermute(outs, inv_idx, h, w):
    """outs: list of 4 [C, L] -> sum of un-permuted direction outputs."""
    c = outs[0].shape[0]
    row_f = outs[0].reshape(c, h, w)
    col_f = outs[1].reshape(c, w, h).transpose(0, 2, 1)
    diag_f = outs[2][:, inv_idx].reshape(c, h, w)
    anti_f = outs[3][:, inv_idx].reshape(c, h, w)[:, :, ::-1]
    return row_f + col_f + diag_f + anti_f


def _pack_convd(cw):
    """Per d-block, per tap: diag(conv_w[:, k]) as bf16 PE weights."""
    out = np.zeros((ND, DC, 128, 128), np.float32)
    for db in range(ND):
        for k in range(DC):
            np.fill_diagonal(out[db, k], cw[db * 128:(db + 1) * 128, k])
    return out.astype(BF16)


def _pack_wx(wx):
    """Pad W_x columns so dt/B/C rows land at PSUM partitions 0/32/64."""
    out = np.zeros((DI, 96), np.float32)
    out[:, 0:DTR] = wx[:, 0:DTR]
    out[:, 32:32 + DS] = wx[:, DTR:DTR + DS]
    out[:, 64:64 + DS] = wx[:, DTR + DS:]
    return out.astype(BF16)


def kernel(x, W_in, conv_w, conv_b, W_x, W_dt, b_dt, A_log, D_skip, W_out,
           idx, inv_idx):
    x = np.asarray(x, np.float32)
    idx = np.asarray(idx, np.int32)
    inv_idx = np.asarray(inv_idx, np.int32)
    A = -np.exp(np.asarray(A_log, np.float32))        # [4, DI, DS]
    conv_b = np.asarray(conv_b, np.float32)
    b_dt = np.asarray(b_dt, np.float32)
    D_skip = np.asarray(D_skip, np.float32)

    nc = _get_nc()
    in_maps = []
    for core in range(N_CORES):
        d = core // 2      # direction
        bh = core % 2      # batch half
        toks = np.empty((NB, C_, L), BF16)
        for jb in range(NB):
            b = bh * NB + jb
            toks[jb] = _permute_toks(x[b], idx)[d].astype(BF16)
        in_maps.append(dict(
            tokT=toks,
            Win=np.asarray(W_in[d], np.float32).astype(BF16),
            convd=_pack_convd(np.asarray(conv_w[d], np.float32)),
            convb=np.ascontiguousarray(conv_b[d].reshape(DI, 1)),
            Wx=_pack_wx(np.asarray(W_x[d], np.float32)),
            Wdt=np.asarray(W_dt[d], np.float32).astype(BF16),
            bdt=np.ascontiguousarray(b_dt[d].reshape(DI, 1)),
            Acoef=np.ascontiguousarray(A[d]),
            Dsk=np.ascontiguousarray(D_skip[d].reshape(DI, 1)),
            Wout=np.asarray(W_out[d], np.float32).astype(BF16),
        ))

    res = run_bass_kernel_spmd(nc, in_maps, list(range(N_CORES)),
                               trace=PROFILE, **PROFILE_KW)
    _CACHE["last_exec_ns"] = res.exec_time_ns
    outs = res.results

    # gather: per batch b, the 4 direction outputs live on cores d*2 + b//2
    acc = np.zeros((B_, C_, H_, W_), np.float32)
    for b in range(B_):
        bh, jb = b // NB, b % NB
        douts = [np.asarray(outs[d * 2 + bh]["outT"][jb], np.float32)
                 for d in range(4)]
        acc[b] = _unpermute(douts, inv_idx, H_, W_)
    gate = 1.0 / (1.0 + np.exp(-0.25 * acc))
    return x * gate


# revision 32
# speedup vs baseline: 1.4419x; 1.0506x over previous
"""AxialCrossMamba Trainium2 kernel.

Sharding: 8 cores = 4 directions x 2 batch-halves. Each core runs one
direction's Mamba block (its own weights) over two batches. Host does the
direction permutations (row/col/diag/anti, c-major [C, L] token layouts),
and the final 4-direction sigmoid gate.

Per batch the pipeline is split into:
  AC: in-proj matmul (PE) -> causal conv (PE) + fused silu (ACT)
      -> x-proj/dt matmuls (PE) -> dt = Ln(1+Exp(.)) (ACT, one table)
      -> u = dt*conv_out, xsd = conv_out*D (DVE) -> all streamed to DRAM
  D:  selective scan: a = exp(dt*A) per-state ACT rows (fp32, boundary
      columns produced from padded dt columns), b = u*B (DVE),
      tensor_tensor_scan over flattened (s,t) with trailing reset/hold
      columns, h*C + tree reduce over s (DVE)
  E:  gate silu(z) mult + out-proj matmul (PE).

All elementwise work stays on Vector: the Pool/GpSimd engine runs these
ops ~8x slower and its SBUF traffic inflates concurrent Vector ops
25-40% (measured), so offloading is a net loss. AC(j=1) is emitted
interleaved into D(j=0) (and E(0) into D(1)) so PE/ACT input work for
the next batch hides under the scan phase.
"""

import sys

for _p in ("/opt/trn_rl_repo", "/root/.axon_site/_ro/trn_rl_repo"):
    if _p not in sys.path:
        sys.path.insert(0, _p)

from contextlib import ExitStack

import numpy as np
import ml_dtypes

import concourse.bass as bass
from concourse import bacc
import concourse.mybir as mybir
import concourse.tile as tile
from concourse.bass_utils import run_bass_kernel_spmd

BF16 = ml_dtypes.bfloat16

# Problem constants
B_, C_, H_, W_ = 4, 192, 64, 64
L = H_ * W_          # 4096 tokens
DS, DC = 16, 4       # d_state, d_conv
DI = 2 * C_          # 384 d_inner
DTR = (C_ + 15) // 16  # 12 dt_rank
NB = 2               # batches per core
ND = DI // 128       # 3 d-blocks
N_CORES = 8

AF = mybir.ActivationFunctionType
ALU = mybir.AluOpType
FP32 = mybir.dt.float32
BF = mybir.dt.bfloat16


def build_nc(L=L, TC=512, SB=4):
    """Build the SPMD single-core program (identical on all 8 cores)."""
    nc = bacc.Bacc("TRN2", debug=False)
    TB = TC + 2

    # ---- DRAM I/O ----
    tokT = nc.dram_tensor("tokT", [NB, C_, L], BF, kind="ExternalInput").ap()
    Win = nc.dram_tensor("Win", [C_, 2 * DI], BF, kind="ExternalInput").ap()
    convd = nc.dram_tensor("convd", [ND, DC, 128, 128], BF, kind="ExternalInput").ap()
    convb = nc.dram_tensor("convb", [DI, 1], FP32, kind="ExternalInput").ap()
    Wx = nc.dram_tensor("Wx", [DI, 96], BF, kind="ExternalInput").ap()
    Wdt = nc.dram_tensor("Wdt", [DTR, DI], BF, kind="ExternalInput").ap()
    bdt = nc.dram_tensor("bdt", [DI, 1], FP32, kind="ExternalInput").ap()
    Acoef = nc.dram_tensor("Acoef", [DI, DS], FP32, kind="ExternalInput").ap()
    Dsk = nc.dram_tensor("Dsk", [DI, 1], FP32, kind="ExternalInput").ap()
    Wout = nc.dram_tensor("Wout", [DI, C_], BF, kind="ExternalInput").ap()
    outT = nc.dram_tensor("outT", [NB, C_, L], FP32, kind="ExternalOutput").ap()
    # scratch
    z_scr = nc.dram_tensor("z_scr", [NB, ND, 128, L], BF, kind="Internal").ap()
    y_scr = nc.dram_tensor("y_scr", [NB, ND, 128, L], BF, kind="Internal").ap()
    bc_scr = nc.dram_tensor("bc_scr", [NB, 2, L // TC, DS * TC], BF, kind="Internal").ap()
    dt_scr = nc.dram_tensor("dt_scr", [NB, ND, 128, (L // TC) * TB], BF, kind="Internal").ap()
    u_scr = nc.dram_tensor("u_scr", [NB, ND, 128, L], BF, kind="Internal").ap()
    xd_scr = nc.dram_tensor("xd_scr", [NB, ND, 128, L], BF, kind="Internal").ap()

    io = dict(tokT=tokT, Win=Win, convd=convd, convb=convb, Wx=Wx, Wdt=Wdt,
              bdt=bdt, Acoef=Acoef, Dsk=Dsk, Wout=Wout, outT=outT,
              z_scr=z_scr, y_scr=y_scr, bc_scr=bc_scr, dt_scr=dt_scr,
              u_scr=u_scr, xd_scr=xd_scr)
    with tile.TileContext(nc) as tc:
        with ExitStack() as ctx:
            _emit(ctx, tc, nc, io, L=L, TC=TC, SB=SB)
    nc.compile()
    return nc


def _emit(ctx, tc, nc, io, *, L, TC, SB):
    tokT, Win, convd, convb, Wx, Wdt, bdt = (
        io["tokT"], io["Win"], io["convd"], io["convb"], io["Wx"], io["Wdt"],
        io["bdt"])
    Acoef, Dsk, Wout, outT = io["Acoef"], io["Dsk"], io["Wout"], io["outT"]
    z_scr, y_scr, bc_scr = io["z_scr"], io["y_scr"], io["bc_scr"]
    dt_scr, u_scr, xd_scr = io["dt_scr"], io["u_scr"], io["xd_scr"]

    P = 128
    NCH = L // TC          # t-chunks
    NSB = DS // SB         # s-blocks
    NN = L // TC           # matmul n-chunks (== NCH)
    NSZ = TC
    TB = TC + 2            # scan block width (data + reset + hold columns)

    # ---- pools ----
    wp = ctx.enter_context(tc.tile_pool(name="weights", bufs=1))
    big = ctx.enter_context(tc.tile_pool(name="big", bufs=2))    # tok
    xip = ctx.enter_context(tc.tile_pool(name="xip", bufs=1))    # conv inputs
    xcp = ctx.enter_context(tc.tile_pool(name="xcp", bufs=2))    # conv chunks
    af32 = ctx.enter_context(tc.tile_pool(name="af32", bufs=3))  # fp32 scan a
    hbf = ctx.enter_context(tc.tile_pool(name="hbf", bufs=3))    # bf16 scan h
    bcp = ctx.enter_context(tc.tile_pool(name="bcp", bufs=4))    # brep/crep
    bcls = ctx.enter_context(tc.tile_pool(name="bcls", bufs=2))  # b_/hcm
    stp = ctx.enter_context(tc.tile_pool(name="stp", bufs=2))    # D streams
    sm = ctx.enter_context(tc.tile_pool(name="small", bufs=2))
    smE = ctx.enter_context(tc.tile_pool(name="smallE", bufs=2))
    pp = ctx.enter_context(tc.tile_pool(name="psum", bufs=2, space="PSUM"))
    pp2 = ctx.enter_context(tc.tile_pool(name="psum2", bufs=2, space="PSUM"))

    # ---- load weights ----
    win0 = wp.tile([P, 2 * DI], BF, tag="win0")
    win1 = wp.tile([C_ - P, 2 * DI], BF, tag="win1")
    nc.sync.dma_start(win0[:], Win[0:P, :])
    nc.sync.dma_start(win1[:], Win[P:C_, :])
    wdt_full = wp.tile([DTR, DI], BF, tag="wdt")
    nc.sync.dma_start(wdt_full[:], Wdt[:])
    wxs, cw3, cb3, bdt3, ac3, dsk3, wo3 = [], [], [], [], [], [], []
    for db in range(ND):
        r = slice(db * P, (db + 1) * P)
        w1 = wp.tile([P, 96], BF, tag=f"wx{db}")
        nc.sync.dma_start(w1[:], Wx[r, :]); wxs.append(w1)
        wconv = []
        for k in range(DC):
            wck = wp.tile([P, P], BF, tag=f"cw{db}_{k}", name=f"cw{db}_{k}")
            nc.sync.dma_start(wck[:], convd[db, k])
            wconv.append(wck)
        cw3.append(wconv)
        w3 = wp.tile([P, 1], FP32, tag=f"cb{db}")
        nc.sync.dma_start(w3[:], convb[r, :]); cb3.append(w3)
        w4 = wp.tile([P, 1], FP32, tag=f"bdt{db}")
        nc.sync.dma_start(w4[:], bdt[r, :]); bdt3.append(w4)
        w5 = wp.tile([P, DS], FP32, tag=f"ac{db}")
        nc.sync.dma_start(w5[:], Acoef[r, :]); ac3.append(w5)
        w6 = wp.tile([P, 1], FP32, tag=f"dsk{db}")
        nc.sync.dma_start(w6[:], Dsk[r, :]); dsk3.append(w6)
        w7 = wp.tile([P, C_], BF, tag=f"wo{db}")
        nc.sync.dma_start(w7[:], Wout[r, :]); wo3.append(w7)

    def emit_AC(j):
        """Input phases for batch j, chunked; yields between n-chunks."""
        tok0 = big.tile([P, L], BF, tag="big")
        tok1 = big.tile([C_ - P, L], BF, tag="big")
        nc.sync.dma_start(tok0[:], tokT[j, 0:P, :])
        nc.sync.dma_start(tok1[:], tokT[j, P:C_, :])
        xis = []
        for db in range(ND):
            xi = xip.tile([P, L + DC], BF, tag=f"xi{db}", name=f"xi{db}")
            nc.scalar.memzero(xi[:, 0:DC])
            xis.append(xi)
        dtl = sm.tile([DTR, L], BF, tag="dtl", bufs=1)
        yield
        for n in range(NN):
            ns = slice(n * NSZ, (n + 1) * NSZ)
            for m in range(2 * DI // P):   # 0..2 -> xi, 3..5 -> z
                mm = slice(m * P, (m + 1) * P)
                ps = pp.tile([P, NSZ], FP32, tag="ps")
                nc.tensor.matmul(ps[:], win0[:, mm], tok0[:, ns],
                                 start=True, stop=False)
                nc.tensor.matmul(ps[:], win1[:, mm], tok1[:, ns],
                                 start=False, stop=True)
                if m < ND:
                    nc.scalar.copy(
                        xis[m][:, DC + n * NSZ: DC + (n + 1) * NSZ], ps[:])
                else:
                    zt = smE.tile([P, NSZ], BF, tag="ztmp", bufs=2)
                    nc.scalar.activation(zt[:], ps[:], AF.Silu)
                    nc.sync.dma_start(z_scr[j, m - ND, :, ns], zt[:])
            # causal conv (PE, diagonal weights) + fused silu per d-block
            xcs = []
            for db in range(ND):
                psc = pp.tile([P, NSZ], FP32, tag="psc")
                for k in range(DC):
                    nc.tensor.matmul(
                        psc[:], cw3[db][k][:],
                        xis[db][:, 1 + k + n * NSZ: 1 + k + n * NSZ + NSZ],
                        start=(k == 0), stop=(k == DC - 1))
                xc = xcp.tile([P, NSZ], BF, tag=f"xc{db}")
                nc.scalar.activation(xc[:], psc[:], AF.Silu, bias=cb3[db])
                xcs.append(xc)
            # dbc = xs @ Wx -> dt rows + B/C rows
            psd = pp2.tile([96, NSZ], FP32, tag="psd")
            for db in range(ND):
                nc.tensor.matmul(psd[:], wxs[db][:], xcs[db][:],
                                 start=(db == 0), stop=(db == ND - 1))
            nc.scalar.copy(dtl[:, ns], psd[0:DTR, :])
            bt = smE.tile([DS, NSZ], BF, tag="bct")
            ct = smE.tile([DS, NSZ], BF, tag="bct")
            nc.scalar.copy(bt[:], psd[32:32 + DS, :])
            nc.scalar.copy(ct[:], psd[64:64 + DS, :])
            nc.sync.dma_start(
                bc_scr[j, 0, n].rearrange("(s t) -> s t", s=DS), bt[:])
            nc.sync.dma_start(
                bc_scr[j, 1, n].rearrange("(s t) -> s t", s=DS), ct[:])
            # dt = softplus(psm + b_dt) as Ln(1+Exp(.)); pad cols become the
            # scan's reset (dt=30 -> a=0) / hold (dt=0 -> a=1) boundaries.
            for db in range(ND):
                psm = pp.tile([P, NSZ], FP32, tag="ps")
                nc.tensor.matmul(psm[:], wdt_full[:, db * P:(db + 1) * P],
                                 dtl[:, ns], start=True, stop=True)
                dtc = xcp.tile([P, TB], BF, tag="dtc", bufs=3)
                nc.scalar.activation(dtc[:, 0:TC], psm[:], AF.Exp,
                                     bias=bdt3[db])
                nc.vector.memset(dtc[:, TC:TC + 1], 1e13)  # ln(1+.) ~30
                nc.vector.memset(dtc[:, TC + 1:TB], 0.0)   # ln(1+0) = 0
                nc.scalar.activation(dtc[:], dtc[:], AF.Ln, bias=1.0)
                nc.sync.dma_start(dt_scr[j, db, :, n * TB:(n + 1) * TB],
                                  dtc[:])
                ut = xcp.tile([P, NSZ], BF, tag="ut")
                nc.vector.tensor_tensor(ut[:], dtc[:, 0:TC], xcs[db][:],
                                        ALU.mult)
                nc.sync.dma_start(u_scr[j, db, :, ns], ut[:])
                xt = xcp.tile([P, NSZ], BF, tag="xt")
                nc.vector.tensor_scalar_mul(xt[:], xcs[db][:], dsk3[db][:])
                nc.sync.dma_start(xd_scr[j, db, :, ns], xt[:])
            yield

    def emit_D(j, feeder=None):
        """Selective scan for batch j; pulls `feeder` once per chunk."""
        # Carries fp32, slot layout [c0..c3, 0]: slots 1..4 shifted into the
        # reset columns in one copy (seg s resets to carry of seg s+1).
        hcarry = {}
        for db in range(ND):
            for sb in range(NSB):
                t_ = sm.tile([P, SB + 1, 1], FP32, name=f"hcr{db}{sb}",
                             tag=f"hcr{db}_{sb}", bufs=1)
                nc.vector.memset(t_[:, SB:SB + 1, :], 0.0)
                hcarry[(db, sb)] = t_
        for ch in range(NCH):
            cs = slice(ch * TC, (ch + 1) * TC)
            dts, uts = [], []
            for db in range(ND):
                d_ = stp.tile([P, TB], BF, tag=f"dts{db}")
                nc.sync.dma_start(d_[:], dt_scr[j, db, :,
                                                ch * TB:(ch + 1) * TB])
                dts.append(d_)
                u_ = stp.tile([P, TC], BF, tag=f"uts{db}")
                nc.sync.dma_start(u_[:], u_scr[j, db, :, cs])
                uts.append(u_)
            ysum = [[] for _ in range(ND)]
            for sb in range(NSB):
                brep = bcp.tile([P, SB, TC], BF, tag="brep")
                crep = bcp.tile([P, SB, TC], BF, tag="crep")
                nc.sync.dma_start(
                    brep[:],
                    bc_scr[j, 0, ch, sb * SB * TC:(sb + 1) * SB * TC]
                    .rearrange("(s t) -> s t", s=SB)
                    .unsqueeze(0).broadcast_to((P, SB, TC)))
                nc.sync.dma_start(
                    crep[:],
                    bc_scr[j, 1, ch, sb * SB * TC:(sb + 1) * SB * TC]
                    .rearrange("(s t) -> s t", s=SB)
                    .unsqueeze(0).broadcast_to((P, SB, TC)))
                for db in range(ND):
                    a_ = af32.tile([P, SB, TB], FP32, tag="a")
                    for s8 in range(SB):
                        s = sb * SB + s8
                        nc.scalar.activation(a_[:, s8, :], dts[db][:],
                                             AF.Exp, scale=ac3[db][:, s:s + 1])
                    b_ = bcls.tile([P, SB, TB], BF, tag="b", bufs=3)
                    uv = uts[db][:].unsqueeze(1).broadcast_to((P, SB, TC))
                    nc.vector.tensor_tensor(b_[:, :, 0:TC], uv, brep[:],
                                            ALU.mult)
                    nc.vector.memset(b_[:, :, TC + 1:TB], 0.0)
                    if ch == 0:
                        nc.vector.memset(b_[:, :, TC:TC + 1], 0.0)
                        init = 0.0
                    else:
                        nc.vector.tensor_copy(b_[:, :, TC:TC + 1],
                                              hcarry[(db, sb)][:, 1:SB + 1, :])
                        init = hcarry[(db, sb)][:, 0:1, :]
                    h_ = hbf.tile([P, SB, TB], BF, tag="h")
                    nc.vector.tensor_tensor_scan(
                        h_[:].rearrange("p s t -> p (s t)"),
                        a_[:].rearrange("p s t -> p (s t)"),
                        b_[:].rearrange("p s t -> p (s t)"),
                        init, ALU.mult, ALU.add)
                    if ch < NCH - 1:
                        nc.vector.tensor_copy(hcarry[(db, sb)][:, 0:SB, :],
                                              h_[:, :, TC - 1:TC])
                    hcm = bcls.tile([P, SB, TC], BF, tag="hcm")
                    nc.vector.tensor_tensor(hcm[:], h_[:, :, 0:TC], crep[:],
                                            ALU.mult)
                    t2 = sm.tile([P, 2, TC], BF, tag="t2", bufs=2)
                    nc.vector.tensor_tensor(t2[:], hcm[:, 0:2, :],
                                            hcm[:, 2:4, :], ALU.add)
                    ysb = sm.tile([P, TC], BF, tag=f"ysb{db}", bufs=2)
                    nc.vector.tensor_tensor(ysb[:], t2[:, 0, :],
                                            t2[:, 1, :], ALU.add)
                    ysum[db].append(ysb)
                    if sb == 1:
                        yA = sm.tile([P, TC], BF, tag=f"yA{db}", bufs=1)
                        nc.vector.tensor_tensor(yA[:], ysum[db][0][:],
                                                ysum[db][1][:], ALU.add)
                        ysum[db] = [yA]
            for db in range(ND):
                xst = stp.tile([P, TC], BF, tag=f"xst{db}")
                nc.sync.dma_start(xst[:], xd_scr[j, db, :, cs])
                yB = sm.tile([P, TC], BF, tag="yB", bufs=2)
                nc.vector.tensor_tensor(yB[:], ysum[db][1][:],
                                        ysum[db][2][:], ALU.add)
                ysd = sm.tile([P, TC], BF, tag="ysd", bufs=2)
                nc.vector.tensor_tensor(ysd[:], ysum[db][0][:], yB[:],
                                        ALU.add)
                yf = sm.tile([P, TC], BF, tag="yf", bufs=2)
                nc.vector.tensor_tensor(yf[:], ysd[:], xst[:], ALU.add)
                nc.sync.dma_start(y_scr[j, db, :, cs], yf[:])
            if feeder is not None:
                next(feeder, None)

    def emit_E(j):
        """Gate + out-proj for batch j; yields between n-chunks."""
        for n in range(NN):
            ns = slice(n * NSZ, (n + 1) * NSZ)
            ygs = []
            for db in range(ND):
                zt = smE.tile([P, NSZ], BF, tag="ze", bufs=2)
                nc.sync.dma_start(zt[:], z_scr[j, db, :, ns])
                yt = smE.tile([P, NSZ], BF, tag="ye", bufs=2)
                nc.sync.dma_start(yt[:], y_scr[j, db, :, ns])
                nc.vector.tensor_tensor(yt[:], yt[:], zt[:], ALU.mult)
                ygs.append(yt)
            for m in range(2):
                msz = P if m == 0 else C_ - P
                mm = slice(m * P, m * P + msz)
                pso = pp2.tile([msz, NSZ], FP32, tag="pso")
                for db in range(ND):
                    nc.tensor.matmul(pso[:], wo3[db][:, mm], ygs[db][:],
                                     start=(db == 0), stop=(db == ND - 1))
                ot = smE.tile([msz, NSZ], FP32, tag="oe", bufs=1)
                nc.scalar.copy(ot[:], pso[:])
                nc.sync.dma_start(outT[j, mm, ns], ot[:])
            yield

    # Schedule: AC0 | D0 + AC1 interleaved | D1 + E0 interleaved | E1.
    for _ in emit_AC(0):
        pass
    ac1 = emit_AC(1)
    emit_D(0, feeder=ac1)
    for _ in ac1:
        pass
    e0 = emit_E(0)
    emit_D(1, feeder=e0)
    for _ in e0:
        pass
    for _ in emit_E(1):
        pass


# ---------------- host side ----------------

_CACHE = {}
PROFILE = False
PROFILE_KW = {}


def _get_nc():
    if "nc" not in _CACHE:
        _CACHE["nc"] = build_nc()
    return _CACHE["nc"]


def _permute_toks(x, idx):
    """x: [C, H, W] fp32 -> 4 direction token maps, each [C, L] (c-major)."""
    c = x.shape[0]
    row = x.reshape(c, -1)
    col = x.transpose(0, 2, 1).reshape(c, -1)
    diag = row[:, idx]
    anti = x[:, :, ::-1].reshape(c, -1)[:, idx]
    return [row, col, diag, anti]


def _unpermute(outs, inv_idx, h, w):
    """outs: list of 4 [C, L] -> sum of un-permuted direction outputs."""
    c = outs[0].shape[0]
    row_f = outs[0].reshape(c, h, w)
    col_f = outs[1].reshape(c, w, h).transpose(0, 2, 1)
    diag_f = outs[2][:, inv_idx].reshape(c, h, w)
    anti_f = outs[3][:, inv_idx].reshape(c, h, w)[:, :, ::-1]
    return row_f + col_f + diag_f + anti_f


def _pack_convd(cw):
    """Per d-block, per tap: diag(conv_w[:, k]) as bf16 PE weights."""
    out = np.zeros((ND, DC, 128, 128), np.float32)
    for db in range(ND):
        for k in range(DC):
            np.fill_diagonal(out[db, k], cw[db * 128:(db + 1) * 128, k])
    return out.astype(BF16)


def _pack_wx(wx):
    """Pad W_x columns so dt/B/C rows land at PSUM partitions 0/32/64."""
    out = np.zeros((DI, 96), np.float32)
    out[:, 0:DTR] = wx[:, 0:DTR]
    out[:, 32:32 + DS] = wx[:, DTR:DTR + DS]
    out[:, 64:64 + DS] = wx[:, DTR + DS:]
    return out.astype(BF16)


def kernel(x, W_in, conv_w, conv_b, W_x, W_dt, b_dt, A_log, D_skip, W_out,
           idx, inv_idx):
    x = np.asarray(x, np.float32)
    idx = np.asarray(idx, np.int32)
    inv_idx = np.asarray(inv_idx, np.int32)
    A = -np.exp(np.asarray(A_log, np.float32))        # [4, DI, DS]
    conv_b = np.asarray(conv_b, np.float32)
    b_dt = np.asarray(b_dt, np.float32)
    D_skip = np.asarray(D_skip, np.float32)

    nc = _get_nc()
    in_maps = []
    for core in range(N_CORES):
        d = core // 2      # direction
        bh = core % 2      # batch half
        toks = np.empty((NB, C_, L), BF16)
        for jb in range(NB):
            b = bh * NB + jb
            toks[jb] = _permute_toks(x[b], idx)[d].astype(BF16)
        in_maps.append(dict(
            tokT=toks,
            Win=np.asarray(W_in[d], np.float32).astype(BF16),
            convd=_pack_convd(np.asarray(conv_w[d], np.float32)),
            convb=np.ascontiguousarray(conv_b[d].reshape(DI, 1)),
            Wx=_pack_wx(np.asarray(W_x[d], np.float32)),
            Wdt=np.asarray(W_dt[d], np.float32).astype(BF16),
            bdt=np.ascontiguousarray(b_dt[d].reshape(DI, 1)),
            Acoef=np.ascontiguousarray(A[d]),
            Dsk=np.ascontiguousarray(D_skip[d].reshape(DI, 1)),
            Wout=np.asarray(W_out[d], np.float32).astype(BF16),
        ))

    res = run_bass_kernel_spmd(nc, in_maps, list(range(N_CORES)),
                               trace=PROFILE, **PROFILE_KW)
    _CACHE["last_exec_ns"] = res.exec_time_ns
    outs = res.results

    # gather: per batch b, the 4 direction outputs live on cores d*2 + b//2
    acc = np.zeros((B_, C_, H_, W_), np.float32)
    for b in range(B_):
        bh, jb = b // NB, b % NB
        douts = [np.asarray(outs[d * 2 + bh]["outT"][jb], np.float32)
                 for d in range(4)]
        acc[b] = _unpermute(douts, inv_idx, H_, W_)
    gate = 1.0 / (1.0 + np.exp(-0.25 * acc))
    return x * gate


# revision 35
# speedup vs baseline: 1.5063x; 1.0447x over previous
"""AxialCrossMamba Trainium2 kernel.

Sharding: 8 cores = 4 directions x 2 batch-halves. Each core runs one
direction's Mamba block (its own weights) over two batches. Host does the
direction permutations (row/col/diag/anti, c-major [C, L] token layouts),
and the final 4-direction sigmoid gate.

Per batch the pipeline is split into:
  AC: in-proj matmul (PE) -> causal conv (PE) + fused silu (ACT)
      -> x-proj/dt matmuls (PE) -> dt = Ln(1+Exp(.)) (ACT, one table)
      -> u = dt*conv_out, xsd = conv_out*D (DVE) -> all streamed to DRAM
  D:  selective scan: a = exp(dt*A) per-state ACT rows (fp32, boundary
      columns produced from padded dt columns), b = u*B (DVE),
      tensor_tensor_scan over flattened (s,t) with trailing reset/hold
      columns, h*C + tree reduce over s (DVE)
  E:  gate silu(z) mult + out-proj matmul (PE).

All elementwise work stays on Vector: the Pool/GpSimd engine runs these
ops ~8x slower and its SBUF traffic inflates concurrent Vector ops
25-40% (measured), so offloading is a net loss. AC(j=1) is emitted
interleaved into D(j=0) (and E(0) into D(1)) so PE/ACT input work for
the next batch hides under the scan phase.
"""

import sys

for _p in ("/opt/trn_rl_repo", "/root/.axon_site/_ro/trn_rl_repo"):
    if _p not in sys.path:
        sys.path.insert(0, _p)

from contextlib import ExitStack

import numpy as np
import ml_dtypes

import concourse.bass as bass
from concourse import bacc
import concourse.mybir as mybir
import concourse.tile as tile
from concourse.bass_utils import run_bass_kernel_spmd

BF16 = ml_dtypes.bfloat16

# Problem constants
B_, C_, H_, W_ = 4, 192, 64, 64
L = H_ * W_          # 4096 tokens
DS, DC = 16, 4       # d_state, d_conv
DI = 2 * C_          # 384 d_inner
DTR = (C_ + 15) // 16  # 12 dt_rank
NB = 2               # batches per core
ND = DI // 128       # 3 d-blocks
N_CORES = 8

AF = mybir.ActivationFunctionType
ALU = mybir.AluOpType
FP32 = mybir.dt.float32
BF = mybir.dt.bfloat16


def build_nc(L=L, TC=512, SB=4):
    """Build the SPMD single-core program (identical on all 8 cores)."""
    nc = bacc.Bacc("TRN2", debug=False)
    TB = TC + 2

    # ---- DRAM I/O ----
    tokT = nc.dram_tensor("tokT", [NB, C_, L], BF, kind="ExternalInput").ap()
    Win = nc.dram_tensor("Win", [C_, 2 * DI], BF, kind="ExternalInput").ap()
    convd = nc.dram_tensor("convd", [ND, DC, 128, 128], BF, kind="ExternalInput").ap()
    convb = nc.dram_tensor("convb", [DI, 1], FP32, kind="ExternalInput").ap()
    Wx = nc.dram_tensor("Wx", [DI, 96], BF, kind="ExternalInput").ap()
    Wdt = nc.dram_tensor("Wdt", [DTR, DI], BF, kind="ExternalInput").ap()
    bdt = nc.dram_tensor("bdt", [DI, 1], FP32, kind="ExternalInput").ap()
    Acoef = nc.dram_tensor("Acoef", [DI, DS], FP32, kind="ExternalInput").ap()
    Dsk = nc.dram_tensor("Dsk", [DI, 1], FP32, kind="ExternalInput").ap()
    Wout = nc.dram_tensor("Wout", [DI, C_], BF, kind="ExternalInput").ap()
    outT = nc.dram_tensor("outT", [NB, C_, L], FP32, kind="ExternalOutput").ap()
    # scratch
    z_scr = nc.dram_tensor("z_scr", [NB, ND, 128, L], BF, kind="Internal").ap()
    y_scr = nc.dram_tensor("y_scr", [NB, ND, 128, L], BF, kind="Internal").ap()
    bc_scr = nc.dram_tensor("bc_scr", [NB, 2, L // TC, DS * TC], BF, kind="Internal").ap()
    dt_scr = nc.dram_tensor("dt_scr", [NB, ND, 128, (L // TC) * TB], BF, kind="Internal").ap()
    u_scr = nc.dram_tensor("u_scr", [NB, ND, 128, L], BF, kind="Internal").ap()
    xd_scr = nc.dram_tensor("xd_scr", [NB, ND, 128, L], BF, kind="Internal").ap()

    io = dict(tokT=tokT, Win=Win, convd=convd, convb=convb, Wx=Wx, Wdt=Wdt,
              bdt=bdt, Acoef=Acoef, Dsk=Dsk, Wout=Wout, outT=outT,
              z_scr=z_scr, y_scr=y_scr, bc_scr=bc_scr, dt_scr=dt_scr,
              u_scr=u_scr, xd_scr=xd_scr)
    with tile.TileContext(nc) as tc:
        with ExitStack() as ctx:
            _emit(ctx, tc, nc, io, L=L, TC=TC, SB=SB)
    nc.compile()
    return nc


def _emit(ctx, tc, nc, io, *, L, TC, SB):
    tokT, Win, convd, convb, Wx, Wdt, bdt = (
        io["tokT"], io["Win"], io["convd"], io["convb"], io["Wx"], io["Wdt"],
        io["bdt"])
    Acoef, Dsk, Wout, outT = io["Acoef"], io["Dsk"], io["Wout"], io["outT"]
    z_scr, y_scr, bc_scr = io["z_scr"], io["y_scr"], io["bc_scr"]
    dt_scr, u_scr, xd_scr = io["dt_scr"], io["u_scr"], io["xd_scr"]

    P = 128
    NCH = L // TC          # t-chunks
    NSB = DS // SB         # s-blocks
    NN = L // TC           # matmul n-chunks (== NCH)
    NSZ = TC
    TB = TC + 2            # scan block width (data + reset + hold columns)

    # ---- pools ----
    wp = ctx.enter_context(tc.tile_pool(name="weights", bufs=1))
    big = ctx.enter_context(tc.tile_pool(name="big", bufs=2))    # tok
    xip = ctx.enter_context(tc.tile_pool(name="xip", bufs=1))    # conv inputs
    xcp = ctx.enter_context(tc.tile_pool(name="xcp", bufs=2))    # conv chunks
    af32 = ctx.enter_context(tc.tile_pool(name="af32", bufs=3))  # fp32 scan a
    hbf = ctx.enter_context(tc.tile_pool(name="hbf", bufs=3))    # bf16 scan h
    bcp = ctx.enter_context(tc.tile_pool(name="bcp", bufs=4))    # brep/crep
    bcls = ctx.enter_context(tc.tile_pool(name="bcls", bufs=2))  # b_/hcm
    stp = ctx.enter_context(tc.tile_pool(name="stp", bufs=2))    # D streams
    sm = ctx.enter_context(tc.tile_pool(name="small", bufs=2))
    smE = ctx.enter_context(tc.tile_pool(name="smallE", bufs=2))
    pp = ctx.enter_context(tc.tile_pool(name="psum", bufs=2, space="PSUM"))
    pp2 = ctx.enter_context(tc.tile_pool(name="psum2", bufs=2, space="PSUM"))

    # ---- load weights ----
    win0 = wp.tile([P, 2 * DI], BF, tag="win0")
    win1 = wp.tile([C_ - P, 2 * DI], BF, tag="win1")
    nc.sync.dma_start(win0[:], Win[0:P, :])
    nc.sync.dma_start(win1[:], Win[P:C_, :])
    wdt_full = wp.tile([DTR, DI], BF, tag="wdt")
    nc.sync.dma_start(wdt_full[:], Wdt[:])
    wxs, cw3, cb3, bdt3, ac3, dsk3, wo3 = [], [], [], [], [], [], []
    for db in range(ND):
        r = slice(db * P, (db + 1) * P)
        w1 = wp.tile([P, 96], BF, tag=f"wx{db}")
        nc.sync.dma_start(w1[:], Wx[r, :]); wxs.append(w1)
        wconv = []
        for k in range(DC):
            wck = wp.tile([P, P], BF, tag=f"cw{db}_{k}", name=f"cw{db}_{k}")
            nc.sync.dma_start(wck[:], convd[db, k])
            wconv.append(wck)
        cw3.append(wconv)
        w3 = wp.tile([P, 1], FP32, tag=f"cb{db}")
        nc.sync.dma_start(w3[:], convb[r, :]); cb3.append(w3)
        w4 = wp.tile([P, 1], FP32, tag=f"bdt{db}")
        nc.sync.dma_start(w4[:], bdt[r, :]); bdt3.append(w4)
        w5 = wp.tile([P, DS], FP32, tag=f"ac{db}")
        nc.sync.dma_start(w5[:], Acoef[r, :]); ac3.append(w5)
        w6 = wp.tile([P, 1], FP32, tag=f"dsk{db}")
        nc.sync.dma_start(w6[:], Dsk[r, :]); dsk3.append(w6)
        w7 = wp.tile([P, C_], BF, tag=f"wo{db}")
        nc.sync.dma_start(w7[:], Wout[r, :]); wo3.append(w7)

    def emit_AC(j):
        """Input phases for batch j, chunked; yields between n-chunks."""
        tok0 = big.tile([P, L], BF, tag="big")
        tok1 = big.tile([C_ - P, L], BF, tag="big")
        nc.sync.dma_start(tok0[:], tokT[j, 0:P, :])
        nc.sync.dma_start(tok1[:], tokT[j, P:C_, :])
        xis = []
        for db in range(ND):
            xi = xip.tile([P, L + DC], BF, tag=f"xi{db}", name=f"xi{db}")
            nc.scalar.memzero(xi[:, 0:DC])
            xis.append(xi)
        dtl = sm.tile([DTR, L], BF, tag="dtl", bufs=1)
        yield
        for n in range(NN):
            ns = slice(n * NSZ, (n + 1) * NSZ)
            for m in range(2 * DI // P):   # 0..2 -> xi, 3..5 -> z
                mm = slice(m * P, (m + 1) * P)
                ps = pp.tile([P, NSZ], FP32, tag="ps")
                nc.tensor.matmul(ps[:], win0[:, mm], tok0[:, ns],
                                 start=True, stop=False)
                nc.tensor.matmul(ps[:], win1[:, mm], tok1[:, ns],
                                 start=False, stop=True)
                if m < ND:
                    nc.scalar.copy(
                        xis[m][:, DC + n * NSZ: DC + (n + 1) * NSZ], ps[:])
                else:
                    zt = smE.tile([P, NSZ], BF, tag="ztmp", bufs=2)
                    nc.scalar.activation(zt[:], ps[:], AF.Silu)
                    nc.sync.dma_start(z_scr[j, m - ND, :, ns], zt[:])
            # causal conv (PE, diagonal weights) + fused silu per d-block
            xcs = []
            for db in range(ND):
                psc = pp.tile([P, NSZ], FP32, tag="psc")
                for k in range(DC):
                    nc.tensor.matmul(
                        psc[:], cw3[db][k][:],
                        xis[db][:, 1 + k + n * NSZ: 1 + k + n * NSZ + NSZ],
                        start=(k == 0), stop=(k == DC - 1))
                xc = xcp.tile([P, NSZ], BF, tag=f"xc{db}")
                nc.scalar.activation(xc[:], psc[:], AF.Silu, bias=cb3[db])
                xcs.append(xc)
            # dbc = xs @ Wx -> dt rows + B/C rows
            psd = pp2.tile([96, NSZ], FP32, tag="psd")
            for db in range(ND):
                nc.tensor.matmul(psd[:], wxs[db][:], xcs[db][:],
                                 start=(db == 0), stop=(db == ND - 1))
            nc.scalar.copy(dtl[:, ns], psd[0:DTR, :])
            bt = smE.tile([DS, NSZ], BF, tag="bct")
            ct = smE.tile([DS, NSZ], BF, tag="bct")
            nc.scalar.copy(bt[:], psd[32:32 + DS, :])
            nc.scalar.copy(ct[:], psd[64:64 + DS, :])
            nc.sync.dma_start(
                bc_scr[j, 0, n].rearrange("(s t) -> s t", s=DS), bt[:])
            nc.sync.dma_start(
                bc_scr[j, 1, n].rearrange("(s t) -> s t", s=DS), ct[:])
            # dt = softplus(psm + b_dt) as Ln(1+Exp(.)); pad cols become the
            # scan's reset (dt=30 -> a=0) / hold (dt=0 -> a=1) boundaries.
            for db in range(ND):
                psm = pp.tile([P, NSZ], FP32, tag="ps")
                nc.tensor.matmul(psm[:], wdt_full[:, db * P:(db + 1) * P],
                                 dtl[:, ns], start=True, stop=True)
                dtc = xcp.tile([P, TB], BF, tag="dtc", bufs=3)
                nc.scalar.activation(dtc[:, 0:TC], psm[:], AF.Exp,
                                     bias=bdt3[db])
                nc.vector.memset(dtc[:, TC:TC + 1], 1e13)  # ln(1+.) ~30
                nc.vector.memset(dtc[:, TC + 1:TB], 0.0)   # ln(1+0) = 0
                nc.scalar.activation(dtc[:], dtc[:], AF.Ln, bias=1.0)
                nc.sync.dma_start(dt_scr[j, db, :, n * TB:(n + 1) * TB],
                                  dtc[:])
                ut = xcp.tile([P, NSZ], BF, tag="ut")
                nc.vector.tensor_tensor(ut[:], dtc[:, 0:TC], xcs[db][:],
                                        ALU.mult)
                nc.sync.dma_start(u_scr[j, db, :, ns], ut[:])
                xt = xcp.tile([P, NSZ], BF, tag="xt")
                nc.vector.tensor_scalar_mul(xt[:], xcs[db][:], dsk3[db][:])
                nc.sync.dma_start(xd_scr[j, db, :, ns], xt[:])
            yield

    def emit_D(j, feeder=None, feeds_per_chunk=1):
        """Selective scan for batch j; pulls `feeder` once per chunk."""
        # Carries fp32, slot layout [c0..c3, 0]: slots 1..4 shifted into the
        # reset columns in one copy (seg s resets to carry of seg s+1).
        hcarry = {}
        for db in range(ND):
            for sb in range(NSB):
                t_ = sm.tile([P, SB + 1, 1], FP32, name=f"hcr{db}{sb}",
                             tag=f"hcr{db}_{sb}", bufs=1)
                nc.vector.memset(t_[:, SB:SB + 1, :], 0.0)
                hcarry[(db, sb)] = t_
        for ch in range(NCH):
            cs = slice(ch * TC, (ch + 1) * TC)
            dts, uts = [], []
            for db in range(ND):
                d_ = stp.tile([P, TB], BF, tag=f"dts{db}")
                nc.sync.dma_start(d_[:], dt_scr[j, db, :,
                                                ch * TB:(ch + 1) * TB])
                dts.append(d_)
                u_ = stp.tile([P, TC], BF, tag=f"uts{db}")
                nc.sync.dma_start(u_[:], u_scr[j, db, :, cs])
                uts.append(u_)
            ysum = [[] for _ in range(ND)]
            for sb in range(NSB):
                brep = bcp.tile([P, SB, TC], BF, tag="brep")
                crep = bcp.tile([P, SB, TC], BF, tag="crep")
                nc.sync.dma_start(
                    brep[:],
                    bc_scr[j, 0, ch, sb * SB * TC:(sb + 1) * SB * TC]
                    .rearrange("(s t) -> s t", s=SB)
                    .unsqueeze(0).broadcast_to((P, SB, TC)))
                nc.sync.dma_start(
                    crep[:],
                    bc_scr[j, 1, ch, sb * SB * TC:(sb + 1) * SB * TC]
                    .rearrange("(s t) -> s t", s=SB)
                    .unsqueeze(0).broadcast_to((P, SB, TC)))
                for db in range(ND):
                    a_ = af32.tile([P, SB, TB], FP32, tag="a")
                    for s8 in range(SB):
                        s = sb * SB + s8
                        nc.scalar.activation(a_[:, s8, :], dts[db][:],
                                             AF.Exp, scale=ac3[db][:, s:s + 1])
                    b_ = bcls.tile([P, SB, TB], BF, tag="b", bufs=3)
                    uv = uts[db][:].unsqueeze(1).broadcast_to((P, SB, TC))
                    nc.vector.tensor_tensor(b_[:, :, 0:TC], uv, brep[:],
                                            ALU.mult)
                    nc.vector.memset(b_[:, :, TC + 1:TB], 0.0)
                    if ch == 0:
                        nc.vector.memset(b_[:, :, TC:TC + 1], 0.0)
                        init = 0.0
                    else:
                        nc.vector.tensor_copy(b_[:, :, TC:TC + 1],
                                              hcarry[(db, sb)][:, 1:SB + 1, :])
                        init = hcarry[(db, sb)][:, 0:1, :]
                    h_ = hbf.tile([P, SB, TB], BF, tag="h")
                    nc.vector.tensor_tensor_scan(
                        h_[:].rearrange("p s t -> p (s t)"),
                        a_[:].rearrange("p s t -> p (s t)"),
                        b_[:].rearrange("p s t -> p (s t)"),
                        init, ALU.mult, ALU.add)
                    if ch < NCH - 1:
                        nc.vector.tensor_copy(hcarry[(db, sb)][:, 0:SB, :],
                                              h_[:, :, TC - 1:TC])
                    hcm = bcls.tile([P, SB, TC], BF, tag="hcm")
                    nc.vector.tensor_tensor(hcm[:], h_[:, :, 0:TC], crep[:],
                                            ALU.mult)
                    t2 = sm.tile([P, 2, TC], BF, tag="t2", bufs=2)
                    nc.vector.tensor_tensor(t2[:], hcm[:, 0:2, :],
                                            hcm[:, 2:4, :], ALU.add)
                    ysb = sm.tile([P, TC], BF, tag=f"ysb{db}", bufs=2)
                    nc.vector.tensor_tensor(ysb[:], t2[:, 0, :],
                                            t2[:, 1, :], ALU.add)
                    ysum[db].append(ysb)
                    if sb == 1:
                        yA = sm.tile([P, TC], BF, tag=f"yA{db}", bufs=1)
                        nc.vector.tensor_tensor(yA[:], ysum[db][0][:],
                                                ysum[db][1][:], ALU.add)
                        ysum[db] = [yA]
            for db in range(ND):
                xst = stp.tile([P, TC], BF, tag=f"xst{db}")
                nc.sync.dma_start(xst[:], xd_scr[j, db, :, cs])
                yB = sm.tile([P, TC], BF, tag="yB", bufs=2)
                nc.vector.tensor_tensor(yB[:], ysum[db][1][:],
                                        ysum[db][2][:], ALU.add)
                ysd = sm.tile([P, TC], BF, tag="ysd", bufs=2)
                nc.vector.tensor_tensor(ysd[:], ysum[db][0][:], yB[:],
                                        ALU.add)
                yf = sm.tile([P, TC], BF, tag="yf", bufs=2)
                nc.vector.tensor_tensor(yf[:], ysd[:], xst[:], ALU.add)
                nc.sync.dma_start(y_scr[j, db, :, cs], yf[:])
            if feeder is not None:
                for _ in range(feeds_per_chunk):
                    next(feeder, None)

    def emit_E(j):
        """Gate + out-proj for batch j; yields between n-chunks."""
        for n in range(NN):
            ns = slice(n * NSZ, (n + 1) * NSZ)
            ygs = []
            for db in range(ND):
                zt = smE.tile([P, NSZ], BF, tag="ze", bufs=2)
                nc.sync.dma_start(zt[:], z_scr[j, db, :, ns])
                yt = smE.tile([P, NSZ], BF, tag="ye", bufs=2)
                nc.sync.dma_start(yt[:], y_scr[j, db, :, ns])
                nc.vector.tensor_tensor(yt[:], yt[:], zt[:], ALU.mult)
                ygs.append(yt)
            for m in range(2):
                msz = P if m == 0 else C_ - P
                mm = slice(m * P, m * P + msz)
                pso = pp2.tile([msz, NSZ], FP32, tag="pso")
                for db in range(ND):
                    nc.tensor.matmul(pso[:], wo3[db][:, mm], ygs[db][:],
                                     start=(db == 0), stop=(db == ND - 1))
                ot = smE.tile([msz, NSZ], FP32, tag="oe", bufs=1)
                nc.scalar.copy(ot[:], pso[:])
                nc.sync.dma_start(outT[j, mm, ns], ot[:])
            yield

    # Schedule: AC0 | D0 + AC1 interleaved | D1 + E0/E1 interleaved.
    # E1's chunk n depends only on D1's chunk n, so it chases D1 chunk-aligned.
    def chain(*gens):
        for g in gens:
            yield from g

    for _ in emit_AC(0):
        pass
    ac1 = emit_AC(1)
    emit_D(0, feeder=ac1)
    for _ in ac1:
        pass
    e01 = chain(emit_E(0), emit_E(1))
    emit_D(1, feeder=e01, feeds_per_chunk=2)
    for _ in e01:
        pass


# ---------------- host side ----------------

_CACHE = {}
PROFILE = False
PROFILE_KW = {}


def _get_nc():
    if "nc" not in _CACHE:
        _CACHE["nc"] = build_nc()
    return _CACHE["nc"]


def _permute_toks(x, idx):
    """x: [C, H, W] fp32 -> 4 direction token maps, each [C, L] (c-major)."""
    c = x.shape[0]
    row = x.reshape(c, -1)
    col = x.transpose(0, 2, 1).reshape(c, -1)
    diag = row[:, idx]
    anti = x[:, :, ::-1].reshape(c, -1)[:, idx]
    return [row, col, diag, anti]


def _unpermute(outs, inv_idx, h, w):
    """outs: list of 4 [C, L] -> sum of un-permuted direction outputs."""
    c = outs[0].shape[0]
    row_f = outs[0].reshape(c, h, w)
    col_f = outs[1].reshape(c, w, h).transpose(0, 2, 1)
    diag_f = outs[2][:, inv_idx].reshape(c, h, w)
    anti_f = outs[3][:, inv_idx].reshape(c, h, w)[:, :, ::-1]
    return row_f + col_f + diag_f + anti_f


def _pack_convd(cw):
    """Per d-block, per tap: diag(conv_w[:, k]) as bf16 PE weights."""
    out = np.zeros((ND, DC, 128, 128), np.float32)
    for db in range(ND):
        for k in range(DC):
            np.fill_diagonal(out[db, k], cw[db * 128:(db + 1) * 128, k])
    return out.astype(BF16)


def _pack_wx(wx):
    """Pad W_x columns so dt/B/C rows land at PSUM partitions 0/32/64."""
    out = np.zeros((DI, 96), np.float32)
    out[:, 0:DTR] = wx[:, 0:DTR]
    out[:, 32:32 + DS] = wx[:, DTR:DTR + DS]
    out[:, 64:64 + DS] = wx[:, DTR + DS:]
    return out.astype(BF16)


def kernel(x, W_in, conv_w, conv_b, W_x, W_dt, b_dt, A_log, D_skip, W_out,
           idx, inv_idx):
    x = np.asarray(x, np.float32)
    idx = np.asarray(idx, np.int32)
    inv_idx = np.asarray(inv_idx, np.int32)
    A = -np.exp(np.asarray(A_log, np.float32))        # [4, DI, DS]
    conv_b = np.asarray(conv_b, np.float32)
    b_dt = np.asarray(b_dt, np.float32)
    D_skip = np.asarray(D_skip, np.float32)

    nc = _get_nc()
    in_maps = []
    for core in range(N_CORES):
        d = core // 2      # direction
        bh = core % 2      # batch half
        toks = np.empty((NB, C_, L), BF16)
        for jb in range(NB):
            b = bh * NB + jb
            toks[jb] = _permute_toks(x[b], idx)[d].astype(BF16)
        in_maps.append(dict(
            tokT=toks,
            Win=np.asarray(W_in[d], np.float32).astype(BF16),
            convd=_pack_convd(np.asarray(conv_w[d], np.float32)),
            convb=np.ascontiguousarray(conv_b[d].reshape(DI, 1)),
            Wx=_pack_wx(np.asarray(W_x[d], np.float32)),
            Wdt=np.asarray(W_dt[d], np.float32).astype(BF16),
            bdt=np.ascontiguousarray(b_dt[d].reshape(DI, 1)),
            Acoef=np.ascontiguousarray(A[d]),
            Dsk=np.ascontiguousarray(D_skip[d].reshape(DI, 1)),
            Wout=np.asarray(W_out[d], np.float32).astype(BF16),
        ))

    res = run_bass_kernel_spmd(nc, in_maps, list(range(N_CORES)),
                               trace=PROFILE, **PROFILE_KW)
    _CACHE["last_exec_ns"] = res.exec_time_ns
    outs = res.results

    # gather: per batch b, the 4 direction outputs live on cores d*2 + b//2
    acc = np.zeros((B_, C_, H_, W_), np.float32)
    for b in range(B_):
        bh, jb = b // NB, b % NB
        douts = [np.asarray(outs[d * 2 + bh]["outT"][jb], np.float32)
                 for d in range(4)]
        acc[b] = _unpermute(douts, inv_idx, H_, W_)
    gate = 1.0 / (1.0 + np.exp(-0.25 * acc))
    return x * gate
